# revision 24
# baseline (speedup 1.0000x reference)
"""Trainium2 Bass kernel for nn_CosSimRouter_learnable_pad.

Host: routing (tiny, exact fp32 replication of the reference) + final top-k /
gather. Device (8 NeuronCores, Megatron tensor-parallel): the ExpanderModule
(3 MHA blocks + FFN + 4 LayerNorms + scorer) with MHA heads and FFN hidden dim
sharded across cores, bf16 matmuls with fp32 accumulation, chunked bf16
AllReduce after each out-projection and after ffn_w2, pipelined with compute.

Key scheduling tricks: exact (unpadded) Q-side widths; LayerNorm deferred-apply
so MHA3's Q-projection and ffn_w1 run on pre-norm activations during the
AllReduce, corrected afterwards with the rank-1 LN fixup; final LN + scorer
folded into host math on (sp.z, r, -mu*r).

Self-contained: takes full inputs, returns the full output.
"""

import numpy as np
import ml_dtypes

BF16 = ml_dtypes.bfloat16

GRID = 24
HEADS = 16
D = 4096
HID = 8192
LV = GRID * GRID
LT = 64
GAMMA = 0.065
TEMP = 0.05
EXPAND_RATIO = 0.3
NCORES = 8
DH = D // HEADS            # 256 per head
NH_CORE = HEADS // NCORES  # 2 heads per core
DHC = DH * NH_CORE         # 512 per-core head dims
HIDC = HID // NCORES       # 1024 per-core ffn hidden
NT = D // 128              # 32 D-tiles

LAST_EXEC_NS = None
_CACHE = {}


# ---------------------------------------------------------------- host routing
def _route_np(vf, te, mask):
    """Exact fp32 replication of reference._route (numpy)."""
    vn = vf / np.maximum(np.linalg.norm(vf, axis=-1, keepdims=True), np.float32(1e-8))
    tn = te / np.maximum(np.linalg.norm(te, axis=-1, keepdims=True), np.float32(1e-8))
    cs = np.where(mask, (vn @ tn.T).astype(np.float32), np.float32(0.0))
    m = cs.max(-1) / np.float32(TEMP)
    e = np.exp(m - m.max())
    scores = e / e.sum()
    order = np.argsort(-scores, kind="stable")
    cum = np.cumsum(scores[order])
    thr = max(int((cum <= np.float32(GAMMA)).sum()), 1)
    selected = order[:thr]
    offs = np.array([[i, j] for i in (-1, 0, 1) for j in (-1, 0, 1)
                     if not (i == 0 and j == 0)])
    r = np.clip(selected[:, None] // GRID + offs[None, :, 0], 0, GRID - 1)
    c = np.clip(selected[:, None] % GRID + offs[None, :, 1], 0, GRID - 1)
    uniq = np.unique((r * GRID + c).reshape(-1))
    remained = np.setdiff1d(np.arange(LV), uniq)
    return thr, uniq, remained


def _shuffle(m):
    """[K, N] -> [128, K//128, N] so device tile [:, t, :] = rows t*128..t*128+128."""
    k, n = m.shape
    return np.ascontiguousarray(m.reshape(k // 128, 128, n).transpose(1, 0, 2))


def _pad_t(x, lp):
    """x [L, D] -> shuffled transpose [128, 32, lp] (zero-padded columns)."""
    out = np.zeros((D, lp), x.dtype)
    out[:, : x.shape[0]] = x.T
    return _shuffle(out)


def _colsum_tile(w):
    """w [F, D] bf16 -> [128, F//128] f32 column-sum tile ([p, m] = sum_d w[m*128+p])."""
    s = w.astype(np.float32).sum(1)
    return np.ascontiguousarray(s.reshape(-1, 128).T)


# ---------------------------------------------------------------- bass builder
def _build(lc, lr, ncu, ncr):
    from contextlib import ExitStack
    import concourse.bass as bass
    import concourse.tile as tile
    from concourse import bacc, mybir

    BF = mybir.dt.bfloat16
    F32 = mybir.dt.float32
    AF = mybir.ActivationFunctionType
    RG = [list(range(NCORES))]

    nc = bacc.Bacc("TRN2", target_bir_lowering=False, debug=False,
                   num_devices=NCORES)

    catT = nc.dram_tensor("catT", [128, NT, lc], BF, kind="ExternalInput").ap()
    remT = nc.dram_tensor("remT", [128, NT, lr], BF, kind="ExternalInput").ap()
    wqkv = [nc.dram_tensor(f"wqkv{i}", [8, 128, 8, 768], BF,
                           kind="ExternalInput").ap() for i in range(3)]
    wo = [nc.dram_tensor(f"wo{i}", [8, 128, 4, 512], BF,
                         kind="ExternalInput").ap() for i in range(3)]
    w1t = nc.dram_tensor("w1t", [8, 128, 16, 256], BF, kind="ExternalInput").ap()
    w2t = nc.dram_tensor("w2t", [8, 128, 8, 512], BF, kind="ExternalInput").ap()
    spt = nc.dram_tensor("spt", [128, NT, 1], BF, kind="ExternalInput").ap()
    eb_cat = nc.dram_tensor("eb_cat", [128, 1], F32, kind="ExternalInput").ap()
    eb_rem = nc.dram_tensor("eb_rem", [128, 1], F32, kind="ExternalInput").ap()
    sq3_d = nc.dram_tensor("sq3", [128, 4], F32, kind="ExternalInput").ap()
    sw1_d = nc.dram_tensor("sw1", [128, 8], F32, kind="ExternalInput").ap()
    spz_d = nc.dram_tensor("spz", [1, ncr], F32, kind="ExternalOutput").ap()
    r4_d = nc.dram_tensor("r4", [1, ncr], F32, kind="ExternalOutput").ap()
    nm4_d = nc.dram_tensor("nm4", [1, ncr], F32, kind="ExternalOutput").ap()

    with tile.TileContext(nc) as tc, ExitStack() as ctx:
        sb = ctx.enter_context(tc.tile_pool(name="sb", bufs=1))
        ws = ctx.enter_context(tc.tile_pool(name="ws", bufs=3))
        tp = ctx.enter_context(tc.tile_pool(name="tp", bufs=2))
        ps = ctx.enter_context(tc.tile_pool(name="ps", bufs=6, space="PSUM"))
        pst = ctx.enter_context(tc.tile_pool(name="pst", bufs=2, space="PSUM"))
        dr = ctx.enter_context(tc.tile_pool(name="dr", bufs=1, space="DRAM"))

        ones_bf = sb.tile([128, 1], BF, tag="ones", name="ones_bf")
        nc.vector.memset(ones_bf[:], 1.0)
        ones_row = sb.tile([1, 128], F32, tag="onesr", name="ones_row")
        nc.vector.memset(ones_row[:], 1.0)
        eps_t = sb.tile([1, 1], F32, tag="eps", name="eps_t")
        nc.vector.memset(eps_t[:], 1e-5)

        cat_sb = sb.tile([128, NT, lc], BF, tag="actC", name="cat_sb")
        nc.sync.dma_start(cat_sb[:], catT[:])
        rem_sb = sb.tile([128, NT, lr], BF, tag="actA", name="rem_sb")
        nc.sync.dma_start(rem_sb[:], remT[:])
        ebc_sb = sb.tile([128, 1], F32, tag="ebc", name="ebc_sb")
        nc.sync.dma_start(ebc_sb[:], eb_cat[:])
        ebr_sb = sb.tile([128, 1], F32, tag="ebr", name="ebr_sb")
        nc.sync.dma_start(ebr_sb[:], eb_rem[:])
        sq3_sb = sb.tile([128, 4], F32, tag="sq3", name="sq3_sb")
        nc.sync.dma_start(sq3_sb[:], sq3_d[:])
        sw1_sb = sb.tile([128, 8], F32, tag="sw1", name="sw1_sb")
        nc.sync.dma_start(sw1_sb[:], sw1_d[:])

        def bcast(row_f32, lq, nm):
            """[1, lq] f32 -> psum [128, lq] f32 via K=1 outer-product matmul."""
            pb = ps.tile([128, lq], F32, tag="pbig", name=f"bc{nm}")
            nc.tensor.matmul(pb[:], ones_row[:], row_f32, start=True, stop=True)
            return pb

        def ar_pair(lq, nch, nm):
            tpc = NT // nch
            ins_ = [dr.tile([128, tpc, lq], BF, tag=f"ai{nm}{g}", name=f"ai{nm}{g}")
                    for g in range(nch)]
            outs_ = [dr.tile([128, tpc, lq], BF, tag=f"ao{nm}{g}", name=f"ao{nm}{g}")
                     for g in range(nch)]
            return ins_, outs_

        def stage_and_reduce(t, lq, pps, arins, arouts, nm):
            """Copy psum tile t into the staging buffer; every 4 tiles DMA to the
            AR chunk buffer; when a chunk completes, launch its AllReduce."""
            tpc = NT // len(arins)
            g, t4 = t // 4, t % 4
            if t4 == 0:
                stage_and_reduce.cur = tp.tile([128, 4, lq], BF, tag="abig",
                                               bufs=2, name=f"ab{nm}{g}")
            nc.scalar.copy(stage_and_reduce.cur[:, t4, :], pps[:])
            if t4 == 3:
                c = t // tpc
                off = (g % (tpc // 4)) * 4
                nc.sync.dma_start(arins[c][:, off:off + 4, :],
                                  stage_and_reduce.cur[:])
                if t == (c + 1) * tpc - 1:
                    nc.gpsimd.collective_compute(
                        "AllReduce", mybir.AluOpType.add, replica_groups=RG,
                        ins=[arins[c].opt()], outs=[arouts[c].opt()])

        def mha(widx, xq, lq, xkv, lkp, eb_sb, qfix=None):
            """One TP-sharded MHA block; returns chunked AllReduce output tiles.

            xq: [128, NT, >=lq] tile (q-side rhs sliced to exact lq).
            xkv: [128, NT, lkp] tile (k/v side, lkp padded to x128, eb masks pad).
            qfix: (rb_s, nmrb_s, scol) - Q-proj runs on pre-norm xq; psums get
            the rank-1 LN fixup afterwards.
            """
            nlk = lkp // 128
            qT = tp.tile([128, 4, lq], BF, tag="qT", bufs=1, name=f"qT{widx}")
            kT = tp.tile([128, 4, lkp], BF, tag="kT", bufs=1, name=f"kT{widx}")
            vv = tp.tile([128, nlk, DHC], BF, tag="vv", bufs=1, name=f"vv{widx}")
            # ---- fused QKV projection, weight-streamed in two column groups
            for grp in (0, 1):
                if grp == 0:  # cols 0:768 -> q0..q3, k0, k1
                    pls = [ps.tile([128, lq], F32, tag="pbig",
                                   name=f"pq{widx}_{m}") for m in range(4)]
                    pls += [ps.tile([128, lkp], F32, tag="pbig",
                                    name=f"pk{widx}_{m}") for m in range(2)]
                else:  # cols 768:1536 -> k2, k3, v rows
                    pls = [ps.tile([128, lkp], F32, tag="pbig",
                                   name=f"pk{widx}_{2 + m}") for m in range(2)]
                    pls += [ps.tile([128, DHC], F32, tag="pbig",
                                    name=f"pv{widx}_{m}") for m in range(nlk)]
                for kc in range(4):
                    ch = ws.tile([128, 8, 768], BF, tag="wqkvch", bufs=2, name=f"wc{widx}{grp}{kc}")
                    nc.sync.dma_start(ch[:], wqkv[widx][grp * 4 + kc])
                    for t8 in range(8):
                        t = kc * 8 + t8
                        st, sp_ = (t == 0), (t == 31)
                        if grp == 0:
                            for m in range(4):
                                nc.tensor.matmul(pls[m][:], ch[:, t8, m * 128:(m + 1) * 128],
                                                 xq[:, t, 0:lq], start=st, stop=sp_)
                            for m in range(2):
                                nc.tensor.matmul(pls[4 + m][:],
                                                 ch[:, t8, 512 + m * 128:512 + (m + 1) * 128],
                                                 xkv[:, t, :], start=st, stop=sp_)
                        else:
                            for m in range(2):
                                nc.tensor.matmul(pls[m][:], ch[:, t8, m * 128:(m + 1) * 128],
                                                 xkv[:, t, :], start=st, stop=sp_)
                            for mk in range(nlk):
                                nc.tensor.matmul(pls[2 + mk][:],
                                                 xkv[:, t, mk * 128:(mk + 1) * 128],
                                                 ch[:, t8, 256:768], start=st, stop=sp_)
                if grp == 0:
                    for m in range(4):
                        if qfix is None:
                            nc.scalar.copy(qT[:, m, :], pls[m][:])
                        else:
                            rb_s, nmrb_s, scol = qfix
                            f1 = tp.tile([128, lq], F32, tag="fixt", bufs=2,
                                         name=f"f1q{widx}{m}")
                            nc.vector.tensor_mul(f1[:], pls[m][:], rb_s[:])
                            f2 = tp.tile([128, lq], F32, tag="fixt", bufs=2,
                                         name=f"f2q{widx}{m}")
                            nc.vector.tensor_scalar(
                                out=f2[:], in0=nmrb_s[:], scalar1=scol[:, m:m + 1],
                                scalar2=None, op0=mybir.AluOpType.mult)
                            nc.vector.tensor_add(qT[:, m, :], f1[:], f2[:])
                    for m in range(2):
                        nc.scalar.copy(kT[:, m, :], pls[4 + m][:])
                else:
                    for m in range(2):
                        nc.scalar.copy(kT[:, 2 + m, :], pls[m][:])
                    for mk in range(nlk):
                        nc.scalar.copy(vv[:, mk, :], pls[2 + mk][:])
            # ---- attention per head (softmax without max-subtraction)
            oT = tp.tile([128, 4, lq], BF, tag="oT", bufs=1, name=f"oT{widx}")
            for h in range(NH_CORE):
                expT = tp.tile([128, nlk, lq], BF, tag="expT", bufs=1,
                               name=f"expT{widx}_{h}")
                for lkt in range(nlk):
                    sps = ps.tile([128, lq], F32, tag="pbig", name=f"psc{widx}{h}{lkt}")
                    for td in range(2):
                        nc.tensor.matmul(sps[:],
                                         kT[:, h * 2 + td, lkt * 128:(lkt + 1) * 128],
                                         qT[:, h * 2 + td, :],
                                         start=(td == 0), stop=(td == 1))
                    bias = eb_sb[:] if lkt == nlk - 1 else 0.0
                    nc.scalar.activation(expT[:, lkt, :], sps[:], AF.Exp,
                                         scale=1.0 / 16.0, bias=bias)
                dps = pst.tile([1, lq], F32, tag="pstat", name=f"pd{widx}{h}")
                for lkt in range(nlk):
                    nc.tensor.matmul(dps[:], ones_bf[:], expT[:, lkt, :],
                                     start=(lkt == 0), stop=(lkt == nlk - 1))
                rc = tp.tile([1, lq], F32, tag="recip", bufs=1, name=f"rc{widx}{h}")
                nc.vector.reciprocal(rc[:], dps[:])
                rbp = bcast(rc[:], lq, f"r{widx}{h}")
                rbs = tp.tile([128, lq], F32, tag="rbs", bufs=1, name=f"rbs{widx}{h}")
                nc.scalar.copy(rbs[:], rbp[:])
                for td in range(2):
                    ops_ = ps.tile([128, lq], F32, tag="pbig", name=f"po{widx}{h}{td}")
                    for lkt in range(nlk):
                        nc.tensor.matmul(ops_[:],
                                         vv[:, lkt, h * 256 + td * 128:h * 256 + (td + 1) * 128],
                                         expT[:, lkt, :],
                                         start=(lkt == 0), stop=(lkt == nlk - 1))
                    nc.vector.tensor_mul(oT[:, h * 2 + td, :], ops_[:], rbs[:])
            # ---- out projection (row-parallel) + chunked AllReduce
            arins, arouts = ar_pair(lq, 1 if widx == 0 else 2, f"m{widx}")
            for ci in range(8):
                ch = ws.tile([128, 4, 512], BF, tag="wsmall", bufs=3, name=f"woc{widx}{ci}")
                nc.sync.dma_start(ch[:], wo[widx][ci])
                for tl in range(4):
                    t = ci * 4 + tl
                    pps = ps.tile([128, lq], F32, tag="pbig", name=f"pop{widx}{t}")
                    for td in range(4):
                        nc.tensor.matmul(pps[:], ch[:, td, tl * 128:(tl + 1) * 128],
                                         oT[:, td, :], start=(td == 0), stop=(td == 3))
                    stage_and_reduce(t, lq, pps, arins, arouts, f"m{widx}")
            return arouts

        def ln(base, arouts, lq, out_tag, out_name, lpad=None, mode="inplace",
               fused_base=None, extra_mm=None, want_fix=False, sep_tag=None):
            """z = base + ar (optionally base = z_pre*rb + nb fused from a
            deferred LN); stats accumulate per arriving AllReduce chunk.
            mode: "inplace" (normalize z in place), "separate" (keep z pre-norm,
            write normalized copy to sep_tag tile), "defer" (keep z pre-norm,
            return bf16+f32 row broadcasts for downstream fixup/fusion),
            "none" (z transient, stats only).
            Returns (z, applied, r, nmr, rb16, nb16, rb_s, nmrb_s)."""
            z = None
            if mode != "none":
                zw = lpad if lpad is not None else lq
                z = sb.tile([128, NT, zw], BF, tag=out_tag, name=out_name)
                if zw > lq:
                    nc.vector.memset(z[:, :, lq:zw], 0.0)
            sums = pst.tile([1, lq], F32, tag="pstat", name=f"su{out_name}")
            sqs = pst.tile([1, lq], F32, tag="pstat", name=f"sq{out_name}")
            tpc = NT // len(arouts)
            for g in range(NT // 4):
                arB = tp.tile([128, 4, lq], BF, tag="arB", bufs=2,
                              name=f"arB{out_name}{g}")
                c = (g * 4) // tpc
                off = (g * 4) % tpc
                nc.sync.dma_start(arB[:], arouts[c][:, off:off + 4, :])
                for t4 in range(4):
                    t = g * 4 + t4
                    if mode != "none":
                        zt = z[:, t, 0:lq]
                    else:
                        ztile = tp.tile([128, lq], BF, tag="z4t", bufs=2,
                                        name=f"zt{out_name}{t}")
                        zt = ztile[:]
                    if fused_base is not None:
                        zp, frb, fnb = fused_base
                        fz = tp.tile([128, lq], BF, tag="lnt", bufs=2,
                                     name=f"fz{out_name}{t}")
                        nc.vector.tensor_mul(fz[:], zp[:, t, 0:lq], frb[:])
                        nc.vector.tensor_add(fz[:], fz[:], fnb[:])
                        nc.vector.tensor_add(zt, fz[:], arB[:, t4, :])
                    else:
                        nc.vector.tensor_add(zt, base[:, t, 0:lq], arB[:, t4, :])
                    nc.tensor.matmul(sums[:], ones_bf[:], zt,
                                     start=(t == 0), stop=(t == NT - 1))
                    sq = tp.tile([128, lq], BF, tag="sq", bufs=2,
                                 name=f"q{out_name}{t}")
                    nc.vector.tensor_mul(sq[:], zt, zt)
                    nc.tensor.matmul(sqs[:], ones_bf[:], sq[:],
                                     start=(t == 0), stop=(t == NT - 1))
                    if extra_mm is not None:
                        extra_mm(t, zt)
            mu = tp.tile([1, lq], F32, tag="lns", bufs=4, name=f"mu{out_name}")
            nc.scalar.mul(mu[:], sums[:], 1.0 / D)
            ex2 = tp.tile([1, lq], F32, tag="lns", bufs=4, name=f"e2{out_name}")
            nc.scalar.mul(ex2[:], sqs[:], 1.0 / D)
            tmp = tp.tile([1, lq], F32, tag="lns", bufs=4, name=f"va{out_name}")
            nc.vector.tensor_mul(tmp[:], mu[:], mu[:])
            nc.vector.tensor_sub(tmp[:], ex2[:], tmp[:])
            nc.scalar.activation(tmp[:], tmp[:], AF.Sqrt, bias=eps_t[:])
            r_ = tp.tile([1, lq], F32, tag="lns", bufs=4, name=f"r{out_name}")
            nc.vector.reciprocal(r_[:], tmp[:])
            nmr = mu
            nc.vector.tensor_mul(nmr[:], nmr[:], r_[:])
            nc.scalar.mul(nmr[:], nmr[:], -1.0)
            if mode == "none":
                return None, None, r_, nmr, None, None, None, None
            rbp = bcast(r_[:], lq, f"lr{out_name}")
            rb16 = tp.tile([128, lq], BF, tag="lnb", bufs=2, name=f"rb{out_name}")
            nc.scalar.copy(rb16[:], rbp[:])
            rb_s = None
            if want_fix:
                rb_s = tp.tile([128, lq], F32, tag="lnbf", bufs=2,
                               name=f"rf{out_name}")
                nc.scalar.copy(rb_s[:], rbp[:])
            nbp = bcast(nmr[:], lq, f"ln{out_name}")
            nb16 = tp.tile([128, lq], BF, tag="lnb", bufs=2, name=f"nb{out_name}")
            nc.scalar.copy(nb16[:], nbp[:])
            nmrb_s = None
            if want_fix:
                nmrb_s = tp.tile([128, lq], F32, tag="lnbf", bufs=2,
                                 name=f"nf{out_name}")
                nc.scalar.copy(nmrb_s[:], nbp[:])
            applied = None
            if mode == "inplace":
                for t in range(NT):
                    tm = tp.tile([128, lq], BF, tag="lnt", bufs=2,
                                 name=f"tm{out_name}{t}")
                    nc.vector.tensor_mul(tm[:], z[:, t, 0:lq], rb16[:])
                    nc.vector.tensor_add(z[:, t, 0:lq], tm[:], nb16[:])
                applied = z
            elif mode == "separate":
                applied = sb.tile([128, NT, lq], BF, tag=sep_tag,
                                  name=f"{out_name}_ap")
                for t in range(NT):
                    tm = tp.tile([128, lq], BF, tag="lnt", bufs=2,
                                 name=f"tm{out_name}{t}")
                    nc.vector.tensor_mul(tm[:], z[:, t, 0:lq], rb16[:])
                    nc.vector.tensor_add(applied[:, t, :], tm[:], nb16[:])
            return z, applied, r_, nmr, rb16, nb16, rb_s, nmrb_s

        # branch A (cat) and branch B (rem) are independent up to MHA3
        ar1 = mha(0, cat_sb, ncu, cat_sb, lc, ebc_sb)
        ar2 = mha(1, rem_sb, ncr, rem_sb, lr, ebr_sb)
        # x: applied LN1 output, padded to lc for use as MHA3's k/v side
        _, x_bf, _, _, _, _, _, _ = ln(cat_sb, ar1, ncu, "actD", "x_bf",
                                       lpad=lc, mode="inplace")
        # LN3 deferred: z2 stays pre-norm; MHA3's Q runs on z2 with the rank-1
        # fixup; y itself is never materialized (LN2 fuses it into its adds)
        z2, _, _, _, rb16_3, nb16_3, rb3, nf3 = ln(
            rem_sb, ar2, ncr, "actB", "z2_bf", mode="defer", want_fix=True)
        ar3 = mha(2, z2, ncr, x_bf, lc, ebc_sb, qfix=(rb3, nf3, sq3_sb))
        # LN2: base is the fused y = z2*rb+nb; z3 kept pre-norm for ffn_w1;
        # x2 (applied) materialized separately for the z4 residual
        z3, x2_bf, _, _, _, _, rb2, nf2 = ln(
            None, ar3, ncr, "actD", "z3_bf", mode="separate", sep_tag="actA",
            fused_base=(z2, rb16_3, nb16_3), want_fix=True)

        # ---- FFN (column/row parallel); w1 runs on pre-norm z3 with fixup
        hT = sb.tile([128, HIDC // 128, ncr], BF, tag="hT", name="hT")
        for mp in range(4):
            pls = [ps.tile([128, ncr], F32, tag="pbig", name=f"ph{mp}_{m}")
                   for m in range(2)]
            for kc in range(2):
                ch = ws.tile([128, 16, 256], BF, tag="wsmall", bufs=3, name=f"w1c{mp}{kc}")
                nc.sync.dma_start(ch[:], w1t[mp * 2 + kc])
                for t16 in range(16):
                    t = kc * 16 + t16
                    for ml in range(2):
                        nc.tensor.matmul(pls[ml][:], ch[:, t16, ml * 128:(ml + 1) * 128],
                                         z3[:, t, :], start=(t == 0), stop=(t == 31))
            for ml in range(2):
                m = mp * 2 + ml
                f1 = tp.tile([128, ncr], F32, tag="fixt", bufs=2, name=f"f1h{m}")
                nc.vector.tensor_mul(f1[:], pls[ml][:], rb2[:])
                f2 = tp.tile([128, ncr], F32, tag="fixt", bufs=2, name=f"f2h{m}")
                nc.vector.tensor_scalar(
                    out=f2[:], in0=nf2[:], scalar1=sw1_sb[:, m:m + 1],
                    scalar2=None, op0=mybir.AluOpType.mult)
                nc.vector.tensor_add(f1[:], f1[:], f2[:])
                nc.scalar.activation(hT[:, m, :], f1[:], AF.Gelu)
        ar4ins, ar4outs = ar_pair(ncr, 2, "f")
        for ci in range(8):
            ch = ws.tile([128, 8, 512], BF, tag="wsmall", bufs=3, name=f"w2c{ci}")
            nc.sync.dma_start(ch[:], w2t[ci])
            for tl in range(4):
                t = ci * 4 + tl
                pps = ps.tile([128, ncr], F32, tag="pbig", name=f"pw2{t}")
                for th in range(8):
                    nc.tensor.matmul(pps[:], ch[:, th, tl * 128:(tl + 1) * 128],
                                     hT[:, th, :], start=(th == 0), stop=(th == 7))
                stage_and_reduce(t, ncr, pps, ar4ins, ar4outs, "f")

        # ---- LN4 stats + scorer matmul on pre-norm z (normalized on host)
        sp_sb = sb.tile([128, NT, 1], BF, tag="spt", name="sp_sb")
        nc.sync.dma_start(sp_sb[:], spt[:])
        lps = ps.tile([1, ncr], F32, tag="pbig", name="lps")

        def spz_mm(t, zt):
            nc.tensor.matmul(lps[:], sp_sb[:, t, :], zt,
                             start=(t == 0), stop=(t == NT - 1))

        _, _, r4, nm4, _, _, _, _ = ln(x2_bf, ar4outs, ncr, "", "z4",
                                       mode="none", extra_mm=spz_mm)
        lg = tp.tile([1, ncr], F32, tag="lns", bufs=4, name="lg")
        nc.vector.tensor_copy(lg[:], lps[:])
        nc.sync.dma_start(spz_d[:], lg[:])
        nc.sync.dma_start(r4_d[:], r4[:])
        nc.sync.dma_start(nm4_d[:], nm4[:])

    nc.compile()
    return nc


# ---------------------------------------------------------------- entry point
def kernel(**inputs):
    global LAST_EXEC_NS
    vf = np.asarray(inputs["vision_feature"], np.float32)
    te = np.asarray(inputs["text_embed"], np.float32)
    mask = np.asarray(inputs["attention_mask"])

    thr, uniq, remained = _route_np(vf, te, mask)
    cat = np.concatenate([vf[uniq], te], 0)
    rem = vf[remained]
    ncu, ncr = cat.shape[0], rem.shape[0]
    lc = -(-ncu // 128) * 128
    lr = -(-ncr // 128) * 128

    key = (lc, lr, ncu, ncr)
    if key not in _CACHE:
        _CACHE[key] = _build(*key)
    nc = _CACHE[key]

    catT = _pad_t(cat.astype(BF16), lc)
    remT = _pad_t(rem.astype(BF16), lr)

    def _eb(nvalid, lpad):
        v = nvalid - (lpad // 128 - 1) * 128
        b = np.zeros((128, 1), np.float32)
        b[v:] = -1e5
        return b

    eb_cat = _eb(ncu, lc)
    eb_rem = _eb(ncr, lr)

    in_maps = []
    for c in range(NCORES):
        hs = slice(c * DHC, (c + 1) * DHC)
        m = {"catT": catT, "remT": remT, "eb_cat": eb_cat, "eb_rem": eb_rem,
             "spt": _shuffle(np.ascontiguousarray(
                 np.asarray(inputs["sp_w"], np.float32).T.reshape(D, 1).astype(BF16)))}
        for i, w in enumerate(("sa1_w", "sa2_w", "ca_w")):
            win = np.asarray(inputs[w], np.float32)
            wq, wk, wv = win[:D][hs], win[D:2 * D][hs], win[2 * D:][hs]
            sh = _shuffle(np.ascontiguousarray(
                np.concatenate([wq.T, wk.T, wv.T], 1)).astype(BF16))
            m[f"wqkv{i}"] = np.stack([
                sh[:, kc * 8:(kc + 1) * 8, grp * 768:(grp + 1) * 768]
                for grp in range(2) for kc in range(4)])
            if w == "ca_w":
                m["sq3"] = _colsum_tile(wq.astype(BF16))
        for i, w in enumerate(("sa1_ow", "sa2_ow", "ca_ow")):
            wout = np.asarray(inputs[w], np.float32)
            sh = _shuffle(np.ascontiguousarray(wout[:, hs].T).astype(BF16))
            m[f"wo{i}"] = np.stack([sh[:, :, ci * 512:(ci + 1) * 512]
                                    for ci in range(8)])
        w1c = np.asarray(inputs["ffn_w1"], np.float32)[c * HIDC:(c + 1) * HIDC]
        m["sw1"] = _colsum_tile(w1c.astype(BF16))
        sh = _shuffle(np.ascontiguousarray(w1c.T).astype(BF16))
        m["w1t"] = np.stack([sh[:, kc * 16:(kc + 1) * 16, mp * 256:(mp + 1) * 256]
                             for mp in range(4) for kc in range(2)])
        sh = _shuffle(np.ascontiguousarray(
            np.asarray(inputs["ffn_w2"], np.float32)[:, c * HIDC:(c + 1) * HIDC].T
        ).astype(BF16))
        m["w2t"] = np.stack([sh[:, :, ci * 512:(ci + 1) * 512] for ci in range(8)])
        in_maps.append(m)

    from concourse import bass_utils
    res = bass_utils.run_bass_kernel_spmd(nc, in_maps, core_ids=list(range(NCORES)))
    LAST_EXEC_NS = res.exec_time_ns

    rr = res.results[0]
    spz = np.asarray(rr["spz"], np.float32)[0]
    r4 = np.asarray(rr["r4"], np.float32)[0]
    nm4 = np.asarray(rr["nm4"], np.float32)[0]
    s_sp = np.float32(np.asarray(inputs["sp_w"], np.float32)
                      .astype(BF16).astype(np.float32).sum())
    logit = r4 * spz + s_sp * nm4 + np.float32(inputs["sp_b"][0])
    k = max(int(thr * EXPAND_RATIO), 1)
    gi = np.argsort(-logit, kind="stable")[:k]
    final = np.unique(np.concatenate([uniq, remained[gi]]))
    return vf[final]


# revision 27
# speedup vs baseline: 1.0255x; 1.0255x over previous
"""Trainium2 Bass kernel for nn_CosSimRouter_learnable_pad.

Host: routing (tiny, exact fp32 replication of the reference) + final top-k /
gather. Device (8 NeuronCores, Megatron tensor-parallel): the ExpanderModule
(3 MHA blocks + FFN + 4 LayerNorms + scorer) with MHA heads and FFN hidden dim
sharded across cores, bf16 matmuls with fp32 accumulation, chunked bf16
AllReduce after each out-projection and after ffn_w2, pipelined with compute.

Key scheduling tricks: exact (unpadded) Q-side widths; LayerNorm deferred-apply
so MHA3's Q-projection and ffn_w1 run on pre-norm activations during the
AllReduce, corrected afterwards with the rank-1 LN fixup; final LN + scorer
folded into host math on (sp.z, r, -mu*r).

Self-contained: takes full inputs, returns the full output.
"""

import numpy as np
import ml_dtypes

BF16 = ml_dtypes.bfloat16

GRID = 24
HEADS = 16
D = 4096
HID = 8192
LV = GRID * GRID
LT = 64
GAMMA = 0.065
TEMP = 0.05
EXPAND_RATIO = 0.3
NCORES = 8
DH = D // HEADS            # 256 per head
NH_CORE = HEADS // NCORES  # 2 heads per core
DHC = DH * NH_CORE         # 512 per-core head dims
HIDC = HID // NCORES       # 1024 per-core ffn hidden
NT = D // 128              # 32 D-tiles

LAST_EXEC_NS = None
_CACHE = {}


# ---------------------------------------------------------------- host routing
def _route_np(vf, te, mask):
    """Exact fp32 replication of reference._route (numpy)."""
    vn = vf / np.maximum(np.linalg.norm(vf, axis=-1, keepdims=True), np.float32(1e-8))
    tn = te / np.maximum(np.linalg.norm(te, axis=-1, keepdims=True), np.float32(1e-8))
    cs = np.where(mask, (vn @ tn.T).astype(np.float32), np.float32(0.0))
    m = cs.max(-1) / np.float32(TEMP)
    e = np.exp(m - m.max())
    scores = e / e.sum()
    order = np.argsort(-scores, kind="stable")
    cum = np.cumsum(scores[order])
    thr = max(int((cum <= np.float32(GAMMA)).sum()), 1)
    selected = order[:thr]
    offs = np.array([[i, j] for i in (-1, 0, 1) for j in (-1, 0, 1)
                     if not (i == 0 and j == 0)])
    r = np.clip(selected[:, None] // GRID + offs[None, :, 0], 0, GRID - 1)
    c = np.clip(selected[:, None] % GRID + offs[None, :, 1], 0, GRID - 1)
    uniq = np.unique((r * GRID + c).reshape(-1))
    remained = np.setdiff1d(np.arange(LV), uniq)
    return thr, uniq, remained


def _shuffle(m):
    """[K, N] -> [128, K//128, N] so device tile [:, t, :] = rows t*128..t*128+128."""
    k, n = m.shape
    return np.ascontiguousarray(m.reshape(k // 128, 128, n).transpose(1, 0, 2))


def _pad_t(x, lp):
    """x [L, D] -> shuffled transpose [128, 32, lp] (zero-padded columns)."""
    out = np.zeros((D, lp), x.dtype)
    out[:, : x.shape[0]] = x.T
    return _shuffle(out)


def _colsum_tile(w):
    """w [F, D] bf16 -> [128, F//128] f32 column-sum tile ([p, m] = sum_d w[m*128+p])."""
    s = w.astype(np.float32).sum(1)
    return np.ascontiguousarray(s.reshape(-1, 128).T)


# ---------------------------------------------------------------- bass builder
def _build(lc, lr, ncu, ncr):
    from contextlib import ExitStack
    import concourse.bass as bass
    import concourse.tile as tile
    from concourse import bacc, mybir

    BF = mybir.dt.bfloat16
    F32 = mybir.dt.float32
    AF = mybir.ActivationFunctionType
    RG = [list(range(NCORES))]

    nc = bacc.Bacc("TRN2", target_bir_lowering=False, debug=False,
                   num_devices=NCORES)

    catT = nc.dram_tensor("catT", [128, NT, lc], BF, kind="ExternalInput").ap()
    remT = nc.dram_tensor("remT", [128, NT, lr], BF, kind="ExternalInput").ap()
    wqkv = [nc.dram_tensor(f"wqkv{i}", [8, 128, 8, 768], BF,
                           kind="ExternalInput").ap() for i in range(2)]
    wqkv3 = nc.dram_tensor("wqkv2", [12, 128, 8, 512], BF,
                           kind="ExternalInput").ap()
    wo = [nc.dram_tensor(f"wo{i}", [8, 128, 4, 512], BF,
                         kind="ExternalInput").ap() for i in range(3)]
    w1t = nc.dram_tensor("w1t", [8, 128, 16, 256], BF, kind="ExternalInput").ap()
    w2t = nc.dram_tensor("w2t", [8, 128, 8, 512], BF, kind="ExternalInput").ap()
    spt = nc.dram_tensor("spt", [128, NT, 1], BF, kind="ExternalInput").ap()
    eb_cat = nc.dram_tensor("eb_cat", [128, 1], F32, kind="ExternalInput").ap()
    eb_rem = nc.dram_tensor("eb_rem", [128, 1], F32, kind="ExternalInput").ap()
    sq3_d = nc.dram_tensor("sq3", [128, 4], F32, kind="ExternalInput").ap()
    sw1_d = nc.dram_tensor("sw1", [128, 8], F32, kind="ExternalInput").ap()
    spz_d = nc.dram_tensor("spz", [1, ncr], F32, kind="ExternalOutput").ap()
    r4_d = nc.dram_tensor("r4", [1, ncr], F32, kind="ExternalOutput").ap()
    nm4_d = nc.dram_tensor("nm4", [1, ncr], F32, kind="ExternalOutput").ap()

    with tile.TileContext(nc) as tc, ExitStack() as ctx:
        sb = ctx.enter_context(tc.tile_pool(name="sb", bufs=1))
        ws = ctx.enter_context(tc.tile_pool(name="ws", bufs=3))
        tp = ctx.enter_context(tc.tile_pool(name="tp", bufs=2))
        ps = ctx.enter_context(tc.tile_pool(name="ps", bufs=6, space="PSUM"))
        pst = ctx.enter_context(tc.tile_pool(name="pst", bufs=2, space="PSUM"))
        dr = ctx.enter_context(tc.tile_pool(name="dr", bufs=1, space="DRAM"))

        ones_bf = sb.tile([128, 1], BF, tag="ones", name="ones_bf")
        nc.vector.memset(ones_bf[:], 1.0)
        ones_row = sb.tile([1, 128], F32, tag="onesr", name="ones_row")
        nc.vector.memset(ones_row[:], 1.0)
        eps_t = sb.tile([1, 1], F32, tag="eps", name="eps_t")
        nc.vector.memset(eps_t[:], 1e-5)

        cat_sb = sb.tile([128, NT, lc], BF, tag="actC", name="cat_sb")
        nc.sync.dma_start(cat_sb[:], catT[:])
        rem_sb = sb.tile([128, NT, lr], BF, tag="actA", name="rem_sb")
        nc.sync.dma_start(rem_sb[:], remT[:])
        ebc_sb = sb.tile([128, 1], F32, tag="ebc", name="ebc_sb")
        nc.sync.dma_start(ebc_sb[:], eb_cat[:])
        ebr_sb = sb.tile([128, 1], F32, tag="ebr", name="ebr_sb")
        nc.sync.dma_start(ebr_sb[:], eb_rem[:])
        sq3_sb = sb.tile([128, 4], F32, tag="sq3", name="sq3_sb")
        nc.sync.dma_start(sq3_sb[:], sq3_d[:])
        sw1_sb = sb.tile([128, 8], F32, tag="sw1", name="sw1_sb")
        nc.sync.dma_start(sw1_sb[:], sw1_d[:])

        def bcast(row_f32, lq, nm):
            """[1, lq] f32 -> psum [128, lq] f32 via K=1 outer-product matmul."""
            pb = ps.tile([128, lq], F32, tag="pbig", name=f"bc{nm}")
            nc.tensor.matmul(pb[:], ones_row[:], row_f32, start=True, stop=True)
            return pb

        def ar_pair(lq, nch, nm):
            tpc = NT // nch
            ins_ = [dr.tile([128, tpc, lq], BF, tag=f"ai{nm}{g}", name=f"ai{nm}{g}")
                    for g in range(nch)]
            outs_ = [dr.tile([128, tpc, lq], BF, tag=f"ao{nm}{g}", name=f"ao{nm}{g}")
                     for g in range(nch)]
            return ins_, outs_

        def stage_and_reduce(t, lq, pps, arins, arouts, nm):
            """Copy psum tile t into the staging buffer; every 4 tiles DMA to the
            AR chunk buffer; when a chunk completes, launch its AllReduce."""
            tpc = NT // len(arins)
            g, t4 = t // 4, t % 4
            if t4 == 0:
                stage_and_reduce.cur = tp.tile([128, 4, lq], BF, tag="abig",
                                               bufs=2, name=f"ab{nm}{g}")
            nc.scalar.copy(stage_and_reduce.cur[:, t4, :], pps[:])
            if t4 == 3:
                c = t // tpc
                off = (g % (tpc // 4)) * 4
                nc.sync.dma_start(arins[c][:, off:off + 4, :],
                                  stage_and_reduce.cur[:])
                if t == (c + 1) * tpc - 1:
                    nc.gpsimd.collective_compute(
                        "AllReduce", mybir.AluOpType.add, replica_groups=RG,
                        ins=[arins[c].opt()], outs=[arouts[c].opt()])

        def attention_and_outproj(widx, qT, kT, vv, lq, lkp, eb_sb, wo_d):
            nlk = lkp // 128
            # ---- attention per head (softmax without max-subtraction)
            oT = tp.tile([128, 4, lq], BF, tag="oT", bufs=1, name=f"oT{widx}")
            for h in range(NH_CORE):
                expT = tp.tile([128, nlk, lq], BF, tag="expT", bufs=1,
                               name=f"expT{widx}_{h}")
                for lkt in range(nlk):
                    sps = ps.tile([128, lq], F32, tag="pbig", name=f"psc{widx}{h}{lkt}")
                    for td in range(2):
                        nc.tensor.matmul(sps[:],
                                         kT[:, h * 2 + td, lkt * 128:(lkt + 1) * 128],
                                         qT[:, h * 2 + td, :],
                                         start=(td == 0), stop=(td == 1))
                    bias = eb_sb[:] if lkt == nlk - 1 else 0.0
                    nc.scalar.activation(expT[:, lkt, :], sps[:], AF.Exp,
                                         scale=1.0 / 16.0, bias=bias)
                dps = pst.tile([1, lq], F32, tag="pstat", name=f"pd{widx}{h}")
                for lkt in range(nlk):
                    nc.tensor.matmul(dps[:], ones_bf[:], expT[:, lkt, :],
                                     start=(lkt == 0), stop=(lkt == nlk - 1))
                rc = tp.tile([1, lq], F32, tag="recip", bufs=1, name=f"rc{widx}{h}")
                nc.vector.reciprocal(rc[:], dps[:])
                rbp = bcast(rc[:], lq, f"r{widx}{h}")
                rbs = tp.tile([128, lq], F32, tag="rbs", bufs=1, name=f"rbs{widx}{h}")
                nc.scalar.copy(rbs[:], rbp[:])
                for td in range(2):
                    ops_ = ps.tile([128, lq], F32, tag="pbig", name=f"po{widx}{h}{td}")
                    for lkt in range(nlk):
                        nc.tensor.matmul(ops_[:],
                                         vv[:, lkt, h * 256 + td * 128:h * 256 + (td + 1) * 128],
                                         expT[:, lkt, :],
                                         start=(lkt == 0), stop=(lkt == nlk - 1))
                    nc.vector.tensor_mul(oT[:, h * 2 + td, :], ops_[:], rbs[:])
            # ---- out projection (row-parallel) + chunked AllReduce
            arins, arouts = ar_pair(lq, 1 if widx == 0 else 2, f"m{widx}")
            for ci in range(8):
                ch = ws.tile([128, 4, 512], BF, tag="wsmall", bufs=3, name=f"woc{widx}{ci}")
                nc.sync.dma_start(ch[:], wo_d[ci])
                for tl in range(4):
                    t = ci * 4 + tl
                    pps = ps.tile([128, lq], F32, tag="pbig", name=f"pop{widx}{t}")
                    for td in range(4):
                        nc.tensor.matmul(pps[:], ch[:, td, tl * 128:(tl + 1) * 128],
                                         oT[:, td, :], start=(td == 0), stop=(td == 3))
                    stage_and_reduce(t, lq, pps, arins, arouts, f"m{widx}")
            return arouts

        def mha(widx, xq, lq, xkv, lkp, eb_sb):
            """One TP-sharded MHA block; returns chunked AllReduce output tiles.

            xq: [128, NT, >=lq] tile (q-side rhs sliced to exact lq).
            xkv: [128, NT, lkp] tile (k/v side, lkp padded to x128, eb masks pad).
            """
            nlk = lkp // 128
            qT = tp.tile([128, 4, lq], BF, tag="qT", bufs=1, name=f"qT{widx}")
            kT = tp.tile([128, 4, lkp], BF, tag="kT", bufs=1, name=f"kT{widx}")
            vv = tp.tile([128, nlk, DHC], BF, tag="vv", bufs=1, name=f"vv{widx}")
            # ---- fused QKV projection, weight-streamed in two column groups
            for grp in (0, 1):
                if grp == 0:  # cols 0:768 -> q0..q3, k0, k1
                    pls = [ps.tile([128, lq], F32, tag="pbig",
                                   name=f"pq{widx}_{m}") for m in range(4)]
                    pls += [ps.tile([128, lkp], F32, tag="pbig",
                                    name=f"pk{widx}_{m}") for m in range(2)]
                else:  # cols 768:1536 -> k2, k3, v rows
                    pls = [ps.tile([128, lkp], F32, tag="pbig",
                                   name=f"pk{widx}_{2 + m}") for m in range(2)]
                    pls += [ps.tile([128, DHC], F32, tag="pbig",
                                    name=f"pv{widx}_{m}") for m in range(nlk)]
                for kc in range(4):
                    ch = ws.tile([128, 8, 768], BF, tag="wqkvch", bufs=2, name=f"wc{widx}{grp}{kc}")
                    nc.sync.dma_start(ch[:], wqkv[widx][grp * 4 + kc])
                    for t8 in range(8):
                        t = kc * 8 + t8
                        st, sp_ = (t == 0), (t == 31)
                        if grp == 0:
                            for m in range(4):
                                nc.tensor.matmul(pls[m][:], ch[:, t8, m * 128:(m + 1) * 128],
                                                 xq[:, t, 0:lq], start=st, stop=sp_)
                            for m in range(2):
                                nc.tensor.matmul(pls[4 + m][:],
                                                 ch[:, t8, 512 + m * 128:512 + (m + 1) * 128],
                                                 xkv[:, t, :], start=st, stop=sp_)
                        else:
                            for m in range(2):
                                nc.tensor.matmul(pls[m][:], ch[:, t8, m * 128:(m + 1) * 128],
                                                 xkv[:, t, :], start=st, stop=sp_)
                            for mk in range(nlk):
                                nc.tensor.matmul(pls[2 + mk][:],
                                                 xkv[:, t, mk * 128:(mk + 1) * 128],
                                                 ch[:, t8, 256:768], start=st, stop=sp_)
                if grp == 0:
                    for m in range(4):
                        nc.scalar.copy(qT[:, m, :], pls[m][:])
                    for m in range(2):
                        nc.scalar.copy(kT[:, m, :], pls[4 + m][:])
                else:
                    for m in range(2):
                        nc.scalar.copy(kT[:, 2 + m, :], pls[m][:])
                    for mk in range(nlk):
                        nc.scalar.copy(vv[:, mk, :], pls[2 + mk][:])
            arouts = attention_and_outproj(widx, qT, kT, vv, lq, lkp, eb_sb,
                                            wo[widx])
            return arouts

        def ln(base, arouts, lq, out_tag, out_name, lpad=None, mode="inplace",
               fused_base=None, extra_mm=None, want_fix=False, sep_tag=None,
               hook=None):
            """z = base + ar (optionally base = z_pre*rb + nb fused from a
            deferred LN); stats accumulate per arriving AllReduce chunk.
            mode: "inplace" (normalize z in place), "separate" (keep z pre-norm,
            write normalized copy to sep_tag tile), "defer" (keep z pre-norm,
            return bf16+f32 row broadcasts for downstream fixup/fusion),
            "none" (z transient, stats only).
            Returns (z, applied, r, nmr, rb16, nb16, rb_s, nmrb_s)."""
            z = None
            if mode != "none":
                zw = lpad if lpad is not None else lq
                z = sb.tile([128, NT, zw], BF, tag=out_tag, name=out_name)
                if zw > lq:
                    nc.vector.memset(z[:, :, lq:zw], 0.0)
            sums = pst.tile([1, lq], F32, tag="pstat", name=f"su{out_name}")
            sqs = pst.tile([1, lq], F32, tag="pstat", name=f"sq{out_name}")
            tpc = NT // len(arouts)
            for g in range(NT // 4):
                arB = tp.tile([128, 4, lq], BF, tag="arB", bufs=2,
                              name=f"arB{out_name}{g}")
                c = (g * 4) // tpc
                off = (g * 4) % tpc
                nc.sync.dma_start(arB[:], arouts[c][:, off:off + 4, :])
                for t4 in range(4):
                    t = g * 4 + t4
                    if mode != "none":
                        zt = z[:, t, 0:lq]
                    else:
                        ztile = tp.tile([128, lq], BF, tag="z4t", bufs=2,
                                        name=f"zt{out_name}{t}")
                        zt = ztile[:]
                    if fused_base is not None:
                        zp, frb, fnb = fused_base
                        fz = tp.tile([128, lq], BF, tag="lnt", bufs=2,
                                     name=f"fz{out_name}{t}")
                        nc.vector.tensor_mul(fz[:], zp[:, t, 0:lq], frb[:])
                        nc.vector.tensor_add(fz[:], fz[:], fnb[:])
                        nc.vector.tensor_add(zt, fz[:], arB[:, t4, :])
                    else:
                        nc.vector.tensor_add(zt, base[:, t, 0:lq], arB[:, t4, :])
                    nc.tensor.matmul(sums[:], ones_bf[:], zt,
                                     start=(t == 0), stop=(t == NT - 1))
                    sq = tp.tile([128, lq], BF, tag="sq", bufs=2,
                                 name=f"q{out_name}{t}")
                    nc.vector.tensor_mul(sq[:], zt, zt)
                    nc.tensor.matmul(sqs[:], ones_bf[:], sq[:],
                                     start=(t == 0), stop=(t == NT - 1))
                    if extra_mm is not None:
                        extra_mm(t, zt)
                    if hook is not None:
                        hook(t, zt)
            mu = tp.tile([1, lq], F32, tag="lns", bufs=4, name=f"mu{out_name}")
            nc.scalar.mul(mu[:], sums[:], 1.0 / D)
            ex2 = tp.tile([1, lq], F32, tag="lns", bufs=4, name=f"e2{out_name}")
            nc.scalar.mul(ex2[:], sqs[:], 1.0 / D)
            tmp = tp.tile([1, lq], F32, tag="lns", bufs=4, name=f"va{out_name}")
            nc.vector.tensor_mul(tmp[:], mu[:], mu[:])
            nc.vector.tensor_sub(tmp[:], ex2[:], tmp[:])
            nc.scalar.activation(tmp[:], tmp[:], AF.Sqrt, bias=eps_t[:])
            r_ = tp.tile([1, lq], F32, tag="lns", bufs=4, name=f"r{out_name}")
            nc.vector.reciprocal(r_[:], tmp[:])
            nmr = mu
            nc.vector.tensor_mul(nmr[:], nmr[:], r_[:])
            nc.scalar.mul(nmr[:], nmr[:], -1.0)
            if mode == "none":
                return None, None, r_, nmr, None, None, None, None
            rbp = bcast(r_[:], lq, f"lr{out_name}")
            rb16 = tp.tile([128, lq], BF, tag="lnb", bufs=2, name=f"rb{out_name}")
            nc.scalar.copy(rb16[:], rbp[:])
            rb_s = None
            if want_fix:
                rb_s = tp.tile([128, lq], F32, tag="lnbf", bufs=2,
                               name=f"rf{out_name}")
                nc.scalar.copy(rb_s[:], rbp[:])
            nbp = bcast(nmr[:], lq, f"ln{out_name}")
            nb16 = tp.tile([128, lq], BF, tag="lnb", bufs=2, name=f"nb{out_name}")
            nc.scalar.copy(nb16[:], nbp[:])
            nmrb_s = None
            if want_fix:
                nmrb_s = tp.tile([128, lq], F32, tag="lnbf", bufs=2,
                                 name=f"nf{out_name}")
                nc.scalar.copy(nmrb_s[:], nbp[:])
            applied = None
            if mode == "inplace":
                for t in range(NT):
                    tm = tp.tile([128, lq], BF, tag="lnt", bufs=2,
                                 name=f"tm{out_name}{t}")
                    nc.vector.tensor_mul(tm[:], z[:, t, 0:lq], rb16[:])
                    nc.vector.tensor_add(z[:, t, 0:lq], tm[:], nb16[:])
                applied = z
            elif mode == "separate":
                applied = sb.tile([128, NT, lq], BF, tag=sep_tag,
                                  name=f"{out_name}_ap")
                for t in range(NT):
                    tm = tp.tile([128, lq], BF, tag="lnt", bufs=2,
                                 name=f"tm{out_name}{t}")
                    nc.vector.tensor_mul(tm[:], z[:, t, 0:lq], rb16[:])
                    nc.vector.tensor_add(applied[:, t, :], tm[:], nb16[:])
            return z, applied, r_, nmr, rb16, nb16, rb_s, nmrb_s

        # branch A (cat) and branch B (rem) are independent up to MHA3
        ar1 = mha(0, cat_sb, ncu, cat_sb, lc, ebc_sb)
        ar2 = mha(1, rem_sb, ncr, rem_sb, lr, ebr_sb)
        # x: applied LN1 output, padded to lc for use as MHA3's k/v side
        x_bf, _, _, _, _, _, _, _ = ln(cat_sb, ar1, ncu, "actD", "x_bf",
                                       lpad=lc, mode="inplace")
        x_bf = x_bf  # applied in place

        # ---- MHA3 K/V projection on x (early, independent of AR2)
        nlk3 = lc // 128
        kT3 = tp.tile([128, 4, lc], BF, tag="kT", bufs=1, name="kT3")
        vv3 = tp.tile([128, nlk3, DHC], BF, tag="vv", bufs=1, name="vv3")
        for grp in (1, 2):
            if grp == 1:
                pls3 = [ps.tile([128, lc], F32, tag="pbig", name=f"pk2_{m}")
                        for m in range(4)]
            else:
                pls3 = [ps.tile([128, DHC], F32, tag="pbig", name=f"pv2_{m}")
                        for m in range(nlk3)]
            for kc in range(4):
                ch = ws.tile([128, 8, 512], BF, tag="wqkvch", bufs=2,
                             name=f"wc3{grp}{kc}")
                nc.sync.dma_start(ch[:], wqkv3[grp * 4 + kc])
                for t8 in range(8):
                    t = kc * 8 + t8
                    st, sp_ = (t == 0), (t == 31)
                    if grp == 1:
                        for m in range(4):
                            nc.tensor.matmul(pls3[m][:], ch[:, t8, m * 128:(m + 1) * 128],
                                             x_bf[:, t, :], start=st, stop=sp_)
                    else:
                        for mk in range(nlk3):
                            nc.tensor.matmul(pls3[mk][:],
                                             x_bf[:, t, mk * 128:(mk + 1) * 128],
                                             ch[:, t8, :], start=st, stop=sp_)
            if grp == 1:
                for m in range(4):
                    nc.scalar.copy(kT3[:, m, :], pls3[m][:])
            else:
                for mk in range(nlk3):
                    nc.scalar.copy(vv3[:, mk, :], pls3[mk][:])

        # ---- LN3 (deferred) with MHA3's Q-projection fused into the chunk loop
        q3 = {}

        def q3_hook(t, zt):
            if t == 0:
                q3["p"] = [ps.tile([128, ncr], F32, tag="pbig", name=f"pq2_{m}")
                           for m in range(4)]
            if t % 8 == 0:
                q3["ch"] = ws.tile([128, 8, 512], BF, tag="wqkvch", bufs=2,
                                   name=f"wcq3{t // 8}")
                nc.sync.dma_start(q3["ch"][:], wqkv3[t // 8])
            for m in range(4):
                nc.tensor.matmul(q3["p"][m][:],
                                 q3["ch"][:, t % 8, m * 128:(m + 1) * 128],
                                 zt, start=(t == 0), stop=(t == NT - 1))

        z2, _, _, _, rb16_3, nb16_3, rb3, nf3 = ln(
            rem_sb, ar2, ncr, "actB", "z2_bf", mode="defer", want_fix=True,
            hook=q3_hook)
        qT3 = tp.tile([128, 4, ncr], BF, tag="qT", bufs=1, name="qT3")
        for m in range(4):
            f1 = tp.tile([128, ncr], F32, tag="fixt", bufs=2, name=f"f1q3{m}")
            nc.vector.tensor_mul(f1[:], q3["p"][m][:], rb3[:])
            f2 = tp.tile([128, ncr], F32, tag="fixt", bufs=2, name=f"f2q3{m}")
            nc.vector.tensor_scalar(
                out=f2[:], in0=nf3[:], scalar1=sq3_sb[:, m:m + 1],
                scalar2=None, op0=mybir.AluOpType.mult)
            nc.vector.tensor_add(qT3[:, m, :], f1[:], f2[:])
        ar3 = attention_and_outproj(2, qT3, kT3, vv3, ncr, lc, ebc_sb, wo[2])

        # ---- LN2 with FFN w1 wave-A (hid tiles 0..3) fused into the chunk loop
        hT = sb.tile([128, HIDC // 128, ncr], BF, tag="hT", name="hT")
        w1a = {}

        def w1a_hook(t, zt):
            if t == 0:
                w1a["p"] = [ps.tile([128, ncr], F32, tag="pbig", name=f"ph_{m}")
                            for m in range(4)]
            if t % 16 == 0:
                kc = t // 16
                w1a["ch"] = [ws.tile([128, 16, 256], BF, tag="wsmall", bufs=3,
                                     name=f"w1a{mp}{kc}") for mp in range(2)]
                for mp in range(2):
                    nc.sync.dma_start(w1a["ch"][mp][:], w1t[mp * 2 + kc])
            for mp in range(2):
                for ml in range(2):
                    nc.tensor.matmul(w1a["p"][mp * 2 + ml][:],
                                     w1a["ch"][mp][:, t % 16, ml * 128:(ml + 1) * 128],
                                     zt, start=(t == 0), stop=(t == NT - 1))

        z3, x2_bf, _, _, _, _, rb2, nf2 = ln(
            None, ar3, ncr, "actD", "z3_bf", mode="separate", sep_tag="actA",
            fused_base=(z2, rb16_3, nb16_3), want_fix=True, hook=w1a_hook)

        def w1_fix(m, psrc):
            f1 = tp.tile([128, ncr], F32, tag="fixt", bufs=2, name=f"f1h{m}")
            nc.vector.tensor_mul(f1[:], psrc[:], rb2[:])
            f2 = tp.tile([128, ncr], F32, tag="fixt", bufs=2, name=f"f2h{m}")
            nc.vector.tensor_scalar(
                out=f2[:], in0=nf2[:], scalar1=sw1_sb[:, m:m + 1],
                scalar2=None, op0=mybir.AluOpType.mult)
            nc.vector.tensor_add(f1[:], f1[:], f2[:])
            nc.scalar.activation(hT[:, m, :], f1[:], AF.Gelu)

        for m in range(4):
            w1_fix(m, w1a["p"][m])
        # wave B (hid tiles 4..7) on the completed z3
        for mp in (2, 3):
            plsb = [ps.tile([128, ncr], F32, tag="pbig", name=f"phb{mp}_{m}")
                    for m in range(2)]
            for kc in range(2):
                ch = ws.tile([128, 16, 256], BF, tag="wsmall", bufs=3,
                             name=f"w1b{mp}{kc}")
                nc.sync.dma_start(ch[:], w1t[mp * 2 + kc])
                for t16 in range(16):
                    t = kc * 16 + t16
                    for ml in range(2):
                        nc.tensor.matmul(plsb[ml][:], ch[:, t16, ml * 128:(ml + 1) * 128],
                                         z3[:, t, :], start=(t == 0), stop=(t == 31))
            for ml in range(2):
                w1_fix(mp * 2 + ml, plsb[ml])
        ar4ins, ar4outs = ar_pair(ncr, 2, "f")
        for ci in range(8):
            ch = ws.tile([128, 8, 512], BF, tag="wsmall", bufs=3, name=f"w2c{ci}")
            nc.sync.dma_start(ch[:], w2t[ci])
            for tl in range(4):
                t = ci * 4 + tl
                pps = ps.tile([128, ncr], F32, tag="pbig", name=f"pw2{t}")
                for th in range(8):
                    nc.tensor.matmul(pps[:], ch[:, th, tl * 128:(tl + 1) * 128],
                                     hT[:, th, :], start=(th == 0), stop=(th == 7))
                stage_and_reduce(t, ncr, pps, ar4ins, ar4outs, "f")

        # ---- LN4 stats + scorer matmul on pre-norm z (normalized on host)
        sp_sb = sb.tile([128, NT, 1], BF, tag="spt", name="sp_sb")
        nc.sync.dma_start(sp_sb[:], spt[:])
        lps = ps.tile([1, ncr], F32, tag="pbig", name="lps")

        def spz_mm(t, zt):
            nc.tensor.matmul(lps[:], sp_sb[:, t, :], zt,
                             start=(t == 0), stop=(t == NT - 1))

        _, _, r4, nm4, _, _, _, _ = ln(x2_bf, ar4outs, ncr, "", "z4",
                                       mode="none", extra_mm=spz_mm)
        lg = tp.tile([1, ncr], F32, tag="lns", bufs=4, name="lg")
        nc.vector.tensor_copy(lg[:], lps[:])
        nc.sync.dma_start(spz_d[:], lg[:])
        nc.sync.dma_start(r4_d[:], r4[:])
        nc.sync.dma_start(nm4_d[:], nm4[:])

    nc.compile()
    return nc


# ---------------------------------------------------------------- entry point
def kernel(**inputs):
    global LAST_EXEC_NS
    vf = np.asarray(inputs["vision_feature"], np.float32)
    te = np.asarray(inputs["text_embed"], np.float32)
    mask = np.asarray(inputs["attention_mask"])

    thr, uniq, remained = _route_np(vf, te, mask)
    cat = np.concatenate([vf[uniq], te], 0)
    rem = vf[remained]
    ncu, ncr = cat.shape[0], rem.shape[0]
    lc = -(-ncu // 128) * 128
    lr = -(-ncr // 128) * 128

    key = (lc, lr, ncu, ncr)
    if key not in _CACHE:
        _CACHE[key] = _build(*key)
    nc = _CACHE[key]

    catT = _pad_t(cat.astype(BF16), lc)
    remT = _pad_t(rem.astype(BF16), lr)

    def _eb(nvalid, lpad):
        v = nvalid - (lpad // 128 - 1) * 128
        b = np.zeros((128, 1), np.float32)
        b[v:] = -1e5
        return b

    eb_cat = _eb(ncu, lc)
    eb_rem = _eb(ncr, lr)

    in_maps = []
    for c in range(NCORES):
        hs = slice(c * DHC, (c + 1) * DHC)
        m = {"catT": catT, "remT": remT, "eb_cat": eb_cat, "eb_rem": eb_rem,
             "spt": _shuffle(np.ascontiguousarray(
                 np.asarray(inputs["sp_w"], np.float32).T.reshape(D, 1).astype(BF16)))}
        for i, w in enumerate(("sa1_w", "sa2_w", "ca_w")):
            win = np.asarray(inputs[w], np.float32)
            wq, wk, wv = win[:D][hs], win[D:2 * D][hs], win[2 * D:][hs]
            sh = _shuffle(np.ascontiguousarray(
                np.concatenate([wq.T, wk.T, wv.T], 1)).astype(BF16))
            if w == "ca_w":
                m["wqkv2"] = np.stack([
                    sh[:, kc * 8:(kc + 1) * 8, grp * 512:(grp + 1) * 512]
                    for grp in range(3) for kc in range(4)])
                m["sq3"] = _colsum_tile(wq.astype(BF16))
            else:
                m[f"wqkv{i}"] = np.stack([
                    sh[:, kc * 8:(kc + 1) * 8, grp * 768:(grp + 1) * 768]
                    for grp in range(2) for kc in range(4)])
        for i, w in enumerate(("sa1_ow", "sa2_ow", "ca_ow")):
            wout = np.asarray(inputs[w], np.float32)
            sh = _shuffle(np.ascontiguousarray(wout[:, hs].T).astype(BF16))
            m[f"wo{i}"] = np.stack([sh[:, :, ci * 512:(ci + 1) * 512]
                                    for ci in range(8)])
        w1c = np.asarray(inputs["ffn_w1"], np.float32)[c * HIDC:(c + 1) * HIDC]
        m["sw1"] = _colsum_tile(w1c.astype(BF16))
        sh = _shuffle(np.ascontiguousarray(w1c.T).astype(BF16))
        m["w1t"] = np.stack([sh[:, kc * 16:(kc + 1) * 16, mp * 256:(mp + 1) * 256]
                             for mp in range(4) for kc in range(2)])
        sh = _shuffle(np.ascontiguousarray(
            np.asarray(inputs["ffn_w2"], np.float32)[:, c * HIDC:(c + 1) * HIDC].T
        ).astype(BF16))
        m["w2t"] = np.stack([sh[:, :, ci * 512:(ci + 1) * 512] for ci in range(8)])
        in_maps.append(m)

    from concourse import bass_utils
    res = bass_utils.run_bass_kernel_spmd(nc, in_maps, core_ids=list(range(NCORES)))
    LAST_EXEC_NS = res.exec_time_ns

    rr = res.results[0]
    spz = np.asarray(rr["spz"], np.float32)[0]
    r4 = np.asarray(rr["r4"], np.float32)[0]
    nm4 = np.asarray(rr["nm4"], np.float32)[0]
    s_sp = np.float32(np.asarray(inputs["sp_w"], np.float32)
                      .astype(BF16).astype(np.float32).sum())
    logit = r4 * spz + s_sp * nm4 + np.float32(inputs["sp_b"][0])
    k = max(int(thr * EXPAND_RATIO), 1)
    gi = np.argsort(-logit, kind="stable")[:k]
    final = np.unique(np.concatenate([uniq, remained[gi]]))
    return vf[final]


# revision 28
# speedup vs baseline: 1.0559x; 1.0297x over previous
"""Trainium2 Bass kernel for nn_CosSimRouter_learnable_pad.

Host: routing (tiny, exact fp32 replication of the reference) + final top-k /
gather. Device (8 NeuronCores, Megatron tensor-parallel): the ExpanderModule
(3 MHA blocks + FFN + 4 LayerNorms + scorer) with MHA heads and FFN hidden dim
sharded across cores, bf16 matmuls with fp32 accumulation, chunked bf16
AllReduce after each out-projection and after ffn_w2, pipelined with compute.

Key scheduling tricks: exact (unpadded) Q-side widths; LayerNorm deferred-apply
so MHA3's Q-projection and ffn_w1 run on pre-norm activations during the
AllReduce, corrected afterwards with the rank-1 LN fixup; final LN + scorer
folded into host math on (sp.z, r, -mu*r).

Self-contained: takes full inputs, returns the full output.
"""

import numpy as np
import ml_dtypes

BF16 = ml_dtypes.bfloat16

GRID = 24
HEADS = 16
D = 4096
HID = 8192
LV = GRID * GRID
LT = 64
GAMMA = 0.065
TEMP = 0.05
EXPAND_RATIO = 0.3
NCORES = 8
DH = D // HEADS            # 256 per head
NH_CORE = HEADS // NCORES  # 2 heads per core
DHC = DH * NH_CORE         # 512 per-core head dims
HIDC = HID // NCORES       # 1024 per-core ffn hidden
NT = D // 128              # 32 D-tiles

LAST_EXEC_NS = None
_CACHE = {}


# ---------------------------------------------------------------- host routing
def _route_np(vf, te, mask):
    """Exact fp32 replication of reference._route (numpy)."""
    vn = vf / np.maximum(np.linalg.norm(vf, axis=-1, keepdims=True), np.float32(1e-8))
    tn = te / np.maximum(np.linalg.norm(te, axis=-1, keepdims=True), np.float32(1e-8))
    cs = np.where(mask, (vn @ tn.T).astype(np.float32), np.float32(0.0))
    m = cs.max(-1) / np.float32(TEMP)
    e = np.exp(m - m.max())
    scores = e / e.sum()
    order = np.argsort(-scores, kind="stable")
    cum = np.cumsum(scores[order])
    thr = max(int((cum <= np.float32(GAMMA)).sum()), 1)
    selected = order[:thr]
    offs = np.array([[i, j] for i in (-1, 0, 1) for j in (-1, 0, 1)
                     if not (i == 0 and j == 0)])
    r = np.clip(selected[:, None] // GRID + offs[None, :, 0], 0, GRID - 1)
    c = np.clip(selected[:, None] % GRID + offs[None, :, 1], 0, GRID - 1)
    uniq = np.unique((r * GRID + c).reshape(-1))
    remained = np.setdiff1d(np.arange(LV), uniq)
    return thr, uniq, remained


def _shuffle(m):
    """[K, N] -> [128, K//128, N] so device tile [:, t, :] = rows t*128..t*128+128."""
    k, n = m.shape
    return np.ascontiguousarray(m.reshape(k // 128, 128, n).transpose(1, 0, 2))


def _pad_t(x, lp):
    """x [L, D] -> shuffled transpose [128, 32, lp] (zero-padded columns)."""
    out = np.zeros((D, lp), x.dtype)
    out[:, : x.shape[0]] = x.T
    return _shuffle(out)


def _colsum_tile(w):
    """w [F, D] bf16 -> [128, F//128] f32 column-sum tile ([p, m] = sum_d w[m*128+p])."""
    s = w.astype(np.float32).sum(1)
    return np.ascontiguousarray(s.reshape(-1, 128).T)


# ---------------------------------------------------------------- bass builder
def _build(lc, lr, ncu, ncr):
    from contextlib import ExitStack
    import concourse.bass as bass
    import concourse.tile as tile
    from concourse import bacc, mybir

    BF = mybir.dt.bfloat16
    F32 = mybir.dt.float32
    AF = mybir.ActivationFunctionType
    RG = [list(range(NCORES))]

    nc = bacc.Bacc("TRN2", target_bir_lowering=False, debug=False,
                   num_devices=NCORES)

    catT = nc.dram_tensor("catT", [128, NT, lc], BF, kind="ExternalInput").ap()
    remT = nc.dram_tensor("remT", [128, NT, lr], BF, kind="ExternalInput").ap()
    wqkv = [nc.dram_tensor(f"wqkv{i}", [16, 128, 4, 768], BF,
                           kind="ExternalInput").ap() for i in range(2)]
    wqkv3 = nc.dram_tensor("wqkv2", [24, 128, 4, 512], BF,
                           kind="ExternalInput").ap()
    wo = [nc.dram_tensor(f"wo{i}", [8, 128, 4, 512], BF,
                         kind="ExternalInput").ap() for i in range(3)]
    w1t = nc.dram_tensor("w1t", [8, 128, 16, 256], BF, kind="ExternalInput").ap()
    w2t = nc.dram_tensor("w2t", [8, 128, 8, 512], BF, kind="ExternalInput").ap()
    spt = nc.dram_tensor("spt", [128, NT, 1], BF, kind="ExternalInput").ap()
    eb_cat = nc.dram_tensor("eb_cat", [128, 1], F32, kind="ExternalInput").ap()
    eb_rem = nc.dram_tensor("eb_rem", [128, 1], F32, kind="ExternalInput").ap()
    sq3_d = nc.dram_tensor("sq3", [128, 4], F32, kind="ExternalInput").ap()
    sw1_d = nc.dram_tensor("sw1", [128, 8], F32, kind="ExternalInput").ap()
    spz_d = nc.dram_tensor("spz", [1, ncr], F32, kind="ExternalOutput").ap()
    r4_d = nc.dram_tensor("r4", [1, ncr], F32, kind="ExternalOutput").ap()
    nm4_d = nc.dram_tensor("nm4", [1, ncr], F32, kind="ExternalOutput").ap()

    with tile.TileContext(nc) as tc, ExitStack() as ctx:
        sb = ctx.enter_context(tc.tile_pool(name="sb", bufs=1))
        ws = ctx.enter_context(tc.tile_pool(name="ws", bufs=3))
        tp = ctx.enter_context(tc.tile_pool(name="tp", bufs=2))
        ps = ctx.enter_context(tc.tile_pool(name="ps", bufs=6, space="PSUM"))
        pst = ctx.enter_context(tc.tile_pool(name="pst", bufs=2, space="PSUM"))
        dr = ctx.enter_context(tc.tile_pool(name="dr", bufs=1, space="DRAM"))

        ones_bf = sb.tile([128, 1], BF, tag="ones", name="ones_bf")
        nc.vector.memset(ones_bf[:], 1.0)
        ones_row = sb.tile([1, 128], F32, tag="onesr", name="ones_row")
        nc.vector.memset(ones_row[:], 1.0)
        eps_t = sb.tile([1, 1], F32, tag="eps", name="eps_t")
        nc.vector.memset(eps_t[:], 1e-5)

        cat_sb = sb.tile([128, NT, lc], BF, tag="actC", name="cat_sb")
        nc.sync.dma_start(cat_sb[:], catT[:])
        rem_sb = sb.tile([128, NT, lr], BF, tag="actA", name="rem_sb")
        ebc_sb = sb.tile([128, 1], F32, tag="ebc", name="ebc_sb")
        nc.sync.dma_start(ebc_sb[:], eb_cat[:])
        ebr_sb = sb.tile([128, 1], F32, tag="ebr", name="ebr_sb")
        nc.sync.dma_start(ebr_sb[:], eb_rem[:])
        sq3_sb = sb.tile([128, 4], F32, tag="sq3", name="sq3_sb")
        nc.sync.dma_start(sq3_sb[:], sq3_d[:])
        sw1_sb = sb.tile([128, 8], F32, tag="sw1", name="sw1_sb")
        nc.sync.dma_start(sw1_sb[:], sw1_d[:])

        def bcast(row_f32, lq, nm):
            """[1, lq] f32 -> psum [128, lq] f32 via K=1 outer-product matmul."""
            pb = ps.tile([128, lq], F32, tag="pbig", name=f"bc{nm}")
            nc.tensor.matmul(pb[:], ones_row[:], row_f32, start=True, stop=True)
            return pb

        def ar_pair(lq, nch, nm):
            tpc = NT // nch
            ins_ = [dr.tile([128, tpc, lq], BF, tag=f"ai{nm}{g}", name=f"ai{nm}{g}")
                    for g in range(nch)]
            outs_ = [dr.tile([128, tpc, lq], BF, tag=f"ao{nm}{g}", name=f"ao{nm}{g}")
                     for g in range(nch)]
            return ins_, outs_

        def stage_and_reduce(t, lq, pps, arins, arouts, nm):
            """Copy psum tile t into the staging buffer; every 4 tiles DMA to the
            AR chunk buffer; when a chunk completes, launch its AllReduce."""
            tpc = NT // len(arins)
            g, t4 = t // 4, t % 4
            if t4 == 0:
                stage_and_reduce.cur = tp.tile([128, 4, lq], BF, tag="abig",
                                               bufs=2, name=f"ab{nm}{g}")
            nc.scalar.copy(stage_and_reduce.cur[:, t4, :], pps[:])
            if t4 == 3:
                c = t // tpc
                off = (g % (tpc // 4)) * 4
                nc.sync.dma_start(arins[c][:, off:off + 4, :],
                                  stage_and_reduce.cur[:])
                if t == (c + 1) * tpc - 1:
                    nc.gpsimd.collective_compute(
                        "AllReduce", mybir.AluOpType.add, replica_groups=RG,
                        ins=[arins[c].opt()], outs=[arouts[c].opt()])

        def attention_and_outproj(widx, qT, kT, vv, lq, lkp, eb_sb, wo_d):
            nlk = lkp // 128
            # ---- attention per head (softmax without max-subtraction)
            oT = tp.tile([128, 4, lq], BF, tag="oT", bufs=1, name=f"oT{widx}")
            for h in range(NH_CORE):
                expT = tp.tile([128, nlk, lq], BF, tag="expT", bufs=1,
                               name=f"expT{widx}_{h}")
                for lkt in range(nlk):
                    sps = ps.tile([128, lq], F32, tag="pbig", name=f"psc{widx}{h}{lkt}")
                    for td in range(2):
                        nc.tensor.matmul(sps[:],
                                         kT[:, h * 2 + td, lkt * 128:(lkt + 1) * 128],
                                         qT[:, h * 2 + td, :],
                                         start=(td == 0), stop=(td == 1))
                    bias = eb_sb[:] if lkt == nlk - 1 else 0.0
                    nc.scalar.activation(expT[:, lkt, :], sps[:], AF.Exp,
                                         scale=1.0 / 16.0, bias=bias)
                dps = pst.tile([1, lq], F32, tag="pstat", name=f"pd{widx}{h}")
                for lkt in range(nlk):
                    nc.tensor.matmul(dps[:], ones_bf[:], expT[:, lkt, :],
                                     start=(lkt == 0), stop=(lkt == nlk - 1))
                rc = tp.tile([1, lq], F32, tag="recip", bufs=1, name=f"rc{widx}{h}")
                nc.vector.reciprocal(rc[:], dps[:])
                rbp = bcast(rc[:], lq, f"r{widx}{h}")
                rbs = tp.tile([128, lq], F32, tag="rbs", bufs=1, name=f"rbs{widx}{h}")
                nc.scalar.copy(rbs[:], rbp[:])
                for td in range(2):
                    ops_ = ps.tile([128, lq], F32, tag="pbig", name=f"po{widx}{h}{td}")
                    for lkt in range(nlk):
                        nc.tensor.matmul(ops_[:],
                                         vv[:, lkt, h * 256 + td * 128:h * 256 + (td + 1) * 128],
                                         expT[:, lkt, :],
                                         start=(lkt == 0), stop=(lkt == nlk - 1))
                    nc.vector.tensor_mul(oT[:, h * 2 + td, :], ops_[:], rbs[:])
            # ---- out projection (row-parallel) + chunked AllReduce
            arins, arouts = ar_pair(lq, {0: 1, 1: 2, 2: 4}[widx], f"m{widx}")
            for ci in range(8):
                ch = ws.tile([128, 4, 512], BF, tag="wsmall", bufs=3, name=f"woc{widx}{ci}")
                nc.sync.dma_start(ch[:], wo_d[ci])
                for tl in range(4):
                    t = ci * 4 + tl
                    pps = ps.tile([128, lq], F32, tag="pbig", name=f"pop{widx}{t}")
                    for td in range(4):
                        nc.tensor.matmul(pps[:], ch[:, td, tl * 128:(tl + 1) * 128],
                                         oT[:, td, :], start=(td == 0), stop=(td == 3))
                    stage_and_reduce(t, lq, pps, arins, arouts, f"m{widx}")
            return arouts

        def mha(widx, xq, lq, xkv, lkp, eb_sb):
            """One TP-sharded MHA block; returns chunked AllReduce output tiles.

            xq: [128, NT, >=lq] tile (q-side rhs sliced to exact lq).
            xkv: [128, NT, lkp] tile (k/v side, lkp padded to x128, eb masks pad).
            """
            nlk = lkp // 128
            qT = tp.tile([128, 4, lq], BF, tag="qT", bufs=1, name=f"qT{widx}")
            kT = tp.tile([128, 4, lkp], BF, tag="kT", bufs=1, name=f"kT{widx}")
            vv = tp.tile([128, nlk, DHC], BF, tag="vv", bufs=1, name=f"vv{widx}")
            # ---- fused QKV projection, weight-streamed in two column groups
            for grp in (0, 1):
                if grp == 0:  # cols 0:768 -> q0..q3, k0, k1
                    pls = [ps.tile([128, lq], F32, tag="pbig",
                                   name=f"pq{widx}_{m}") for m in range(4)]
                    pls += [ps.tile([128, lkp], F32, tag="pbig",
                                    name=f"pk{widx}_{m}") for m in range(2)]
                else:  # cols 768:1536 -> k2, k3, v rows
                    pls = [ps.tile([128, lkp], F32, tag="pbig",
                                   name=f"pk{widx}_{2 + m}") for m in range(2)]
                    pls += [ps.tile([128, DHC], F32, tag="pbig",
                                    name=f"pv{widx}_{m}") for m in range(nlk)]
                for kc in range(8):
                    ch = ws.tile([128, 4, 768], BF, tag="wqkvch", bufs=4, name=f"wc{widx}{grp}{kc}")
                    nc.sync.dma_start(ch[:], wqkv[widx][grp * 8 + kc])
                    for t8 in range(4):
                        t = kc * 4 + t8
                        st, sp_ = (t == 0), (t == 31)
                        if grp == 0:
                            for m in range(4):
                                nc.tensor.matmul(pls[m][:], ch[:, t8, m * 128:(m + 1) * 128],
                                                 xq[:, t, 0:lq], start=st, stop=sp_)
                            for m in range(2):
                                nc.tensor.matmul(pls[4 + m][:],
                                                 ch[:, t8, 512 + m * 128:512 + (m + 1) * 128],
                                                 xkv[:, t, :], start=st, stop=sp_)
                        else:
                            for m in range(2):
                                nc.tensor.matmul(pls[m][:], ch[:, t8, m * 128:(m + 1) * 128],
                                                 xkv[:, t, :], start=st, stop=sp_)
                            for mk in range(nlk):
                                nc.tensor.matmul(pls[2 + mk][:],
                                                 xkv[:, t, mk * 128:(mk + 1) * 128],
                                                 ch[:, t8, 256:768], start=st, stop=sp_)
                if grp == 0:
                    for m in range(4):
                        nc.scalar.copy(qT[:, m, :], pls[m][:])
                    for m in range(2):
                        nc.scalar.copy(kT[:, m, :], pls[4 + m][:])
                else:
                    for m in range(2):
                        nc.scalar.copy(kT[:, 2 + m, :], pls[m][:])
                    for mk in range(nlk):
                        nc.scalar.copy(vv[:, mk, :], pls[2 + mk][:])
            arouts = attention_and_outproj(widx, qT, kT, vv, lq, lkp, eb_sb,
                                            wo[widx])
            return arouts

        def ln(base, arouts, lq, out_tag, out_name, lpad=None, mode="inplace",
               fused_base=None, extra_mm=None, want_fix=False, sep_tag=None,
               hook=None):
            """z = base + ar (optionally base = z_pre*rb + nb fused from a
            deferred LN); stats accumulate per arriving AllReduce chunk.
            mode: "inplace" (normalize z in place), "separate" (keep z pre-norm,
            write normalized copy to sep_tag tile), "defer" (keep z pre-norm,
            return bf16+f32 row broadcasts for downstream fixup/fusion),
            "none" (z transient, stats only).
            Returns (z, applied, r, nmr, rb16, nb16, rb_s, nmrb_s)."""
            z = None
            if mode != "none":
                zw = lpad if lpad is not None else lq
                z = sb.tile([128, NT, zw], BF, tag=out_tag, name=out_name)
                if zw > lq:
                    nc.vector.memset(z[:, :, lq:zw], 0.0)
            sums = pst.tile([1, lq], F32, tag="pstat", name=f"su{out_name}")
            sqs = pst.tile([1, lq], F32, tag="pstat", name=f"sq{out_name}")
            tpc = NT // len(arouts)
            for g in range(NT // 4):
                arB = tp.tile([128, 4, lq], BF, tag="arB", bufs=2,
                              name=f"arB{out_name}{g}")
                c = (g * 4) // tpc
                off = (g * 4) % tpc
                nc.sync.dma_start(arB[:], arouts[c][:, off:off + 4, :])
                for t4 in range(4):
                    t = g * 4 + t4
                    if mode != "none":
                        zt = z[:, t, 0:lq]
                    else:
                        ztile = tp.tile([128, lq], BF, tag="z4t", bufs=2,
                                        name=f"zt{out_name}{t}")
                        zt = ztile[:]
                    if fused_base is not None:
                        zp, frb, fnb = fused_base
                        fz = tp.tile([128, lq], BF, tag="lnt", bufs=2,
                                     name=f"fz{out_name}{t}")
                        nc.vector.tensor_mul(fz[:], zp[:, t, 0:lq], frb[:])
                        nc.vector.tensor_add(fz[:], fz[:], fnb[:])
                        nc.vector.tensor_add(zt, fz[:], arB[:, t4, :])
                    else:
                        nc.vector.tensor_add(zt, base[:, t, 0:lq], arB[:, t4, :])
                    nc.tensor.matmul(sums[:], ones_bf[:], zt,
                                     start=(t == 0), stop=(t == NT - 1))
                    sq = tp.tile([128, lq], BF, tag="sq", bufs=2,
                                 name=f"q{out_name}{t}")
                    nc.vector.tensor_mul(sq[:], zt, zt)
                    nc.tensor.matmul(sqs[:], ones_bf[:], sq[:],
                                     start=(t == 0), stop=(t == NT - 1))
                    if extra_mm is not None:
                        extra_mm(t, zt)
                    if hook is not None:
                        hook(t, zt)
            mu = tp.tile([1, lq], F32, tag="lns", bufs=4, name=f"mu{out_name}")
            nc.scalar.mul(mu[:], sums[:], 1.0 / D)
            ex2 = tp.tile([1, lq], F32, tag="lns", bufs=4, name=f"e2{out_name}")
            nc.scalar.mul(ex2[:], sqs[:], 1.0 / D)
            tmp = tp.tile([1, lq], F32, tag="lns", bufs=4, name=f"va{out_name}")
            nc.vector.tensor_mul(tmp[:], mu[:], mu[:])
            nc.vector.tensor_sub(tmp[:], ex2[:], tmp[:])
            nc.scalar.activation(tmp[:], tmp[:], AF.Sqrt, bias=eps_t[:])
            r_ = tp.tile([1, lq], F32, tag="lns", bufs=4, name=f"r{out_name}")
            nc.vector.reciprocal(r_[:], tmp[:])
            nmr = mu
            nc.vector.tensor_mul(nmr[:], nmr[:], r_[:])
            nc.scalar.mul(nmr[:], nmr[:], -1.0)
            if mode == "none":
                return None, None, r_, nmr, None, None, None, None
            rbp = bcast(r_[:], lq, f"lr{out_name}")
            rb16 = tp.tile([128, lq], BF, tag="lnb", bufs=2, name=f"rb{out_name}")
            nc.scalar.copy(rb16[:], rbp[:])
            rb_s = None
            if want_fix:
                rb_s = tp.tile([128, lq], F32, tag="lnbf", bufs=2,
                               name=f"rf{out_name}")
                nc.scalar.copy(rb_s[:], rbp[:])
            nbp = bcast(nmr[:], lq, f"ln{out_name}")
            nb16 = tp.tile([128, lq], BF, tag="lnb", bufs=2, name=f"nb{out_name}")
            nc.scalar.copy(nb16[:], nbp[:])
            nmrb_s = None
            if want_fix:
                nmrb_s = tp.tile([128, lq], F32, tag="lnbf", bufs=2,
                                 name=f"nf{out_name}")
                nc.scalar.copy(nmrb_s[:], nbp[:])
            applied = None
            if mode == "inplace":
                for t in range(NT):
                    tm = tp.tile([128, lq], BF, tag="lnt", bufs=2,
                                 name=f"tm{out_name}{t}")
                    nc.vector.tensor_mul(tm[:], z[:, t, 0:lq], rb16[:])
                    nc.vector.tensor_add(z[:, t, 0:lq], tm[:], nb16[:])
                applied = z
            elif mode == "separate":
                applied = sb.tile([128, NT, lq], BF, tag=sep_tag,
                                  name=f"{out_name}_ap")
                for t in range(NT):
                    tm = tp.tile([128, lq], BF, tag="lnt", bufs=2,
                                 name=f"tm{out_name}{t}")
                    nc.vector.tensor_mul(tm[:], z[:, t, 0:lq], rb16[:])
                    nc.vector.tensor_add(applied[:, t, :], tm[:], nb16[:])
            return z, applied, r_, nmr, rb16, nb16, rb_s, nmrb_s

        # branch A (cat) and branch B (rem) are independent up to MHA3
        ar1 = mha(0, cat_sb, ncu, cat_sb, lc, ebc_sb)
        nc.sync.dma_start(rem_sb[:], remT[:])
        ar2 = mha(1, rem_sb, ncr, rem_sb, lr, ebr_sb)
        # x: applied LN1 output, padded to lc for use as MHA3's k/v side
        x_bf, _, _, _, _, _, _, _ = ln(cat_sb, ar1, ncu, "actD", "x_bf",
                                       lpad=lc, mode="inplace")
        x_bf = x_bf  # applied in place

        # ---- MHA3 K/V projection on x (early, independent of AR2)
        nlk3 = lc // 128
        kT3 = tp.tile([128, 4, lc], BF, tag="kT", bufs=1, name="kT3")
        vv3 = tp.tile([128, nlk3, DHC], BF, tag="vv", bufs=1, name="vv3")
        for grp in (1, 2):
            if grp == 1:
                pls3 = [ps.tile([128, lc], F32, tag="pbig", name=f"pk2_{m}")
                        for m in range(4)]
            else:
                pls3 = [ps.tile([128, DHC], F32, tag="pbig", name=f"pv2_{m}")
                        for m in range(nlk3)]
            for kc in range(8):
                ch = ws.tile([128, 4, 512], BF, tag="wqkvch", bufs=4,
                             name=f"wc3{grp}{kc}")
                nc.sync.dma_start(ch[:], wqkv3[grp * 8 + kc])
                for t8 in range(4):
                    t = kc * 4 + t8
                    st, sp_ = (t == 0), (t == 31)
                    if grp == 1:
                        for m in range(4):
                            nc.tensor.matmul(pls3[m][:], ch[:, t8, m * 128:(m + 1) * 128],
                                             x_bf[:, t, :], start=st, stop=sp_)
                    else:
                        for mk in range(nlk3):
                            nc.tensor.matmul(pls3[mk][:],
                                             x_bf[:, t, mk * 128:(mk + 1) * 128],
                                             ch[:, t8, :], start=st, stop=sp_)
            if grp == 1:
                for m in range(4):
                    nc.scalar.copy(kT3[:, m, :], pls3[m][:])
            else:
                for mk in range(nlk3):
                    nc.scalar.copy(vv3[:, mk, :], pls3[mk][:])

        # ---- LN3 (deferred) with MHA3's Q-projection fused into the chunk loop
        q3 = {}

        def q3_hook(t, zt):
            if t == 0:
                q3["p"] = [ps.tile([128, ncr], F32, tag="pbig", name=f"pq2_{m}")
                           for m in range(4)]
            if t % 4 == 0:
                q3["ch"] = ws.tile([128, 4, 512], BF, tag="wqkvch", bufs=4,
                                   name=f"wcq3{t // 4}")
                nc.sync.dma_start(q3["ch"][:], wqkv3[t // 4])
            for m in range(4):
                nc.tensor.matmul(q3["p"][m][:],
                                 q3["ch"][:, t % 4, m * 128:(m + 1) * 128],
                                 zt, start=(t == 0), stop=(t == NT - 1))

        z2, _, _, _, rb16_3, nb16_3, rb3, nf3 = ln(
            rem_sb, ar2, ncr, "actB", "z2_bf", mode="defer", want_fix=True,
            hook=q3_hook)
        qT3 = tp.tile([128, 4, ncr], BF, tag="qT", bufs=1, name="qT3")
        for m in range(4):
            f1 = tp.tile([128, ncr], F32, tag="fixt", bufs=2, name=f"f1q3{m}")
            nc.vector.tensor_mul(f1[:], q3["p"][m][:], rb3[:])
            f2 = tp.tile([128, ncr], F32, tag="fixt", bufs=2, name=f"f2q3{m}")
            nc.vector.tensor_scalar(
                out=f2[:], in0=nf3[:], scalar1=sq3_sb[:, m:m + 1],
                scalar2=None, op0=mybir.AluOpType.mult)
            nc.vector.tensor_add(qT3[:, m, :], f1[:], f2[:])
        ar3 = attention_and_outproj(2, qT3, kT3, vv3, ncr, lc, ebc_sb, wo[2])

        # ---- LN2 with FFN w1 wave-A (hid tiles 0..3) fused into the chunk loop
        hT = sb.tile([128, HIDC // 128, ncr], BF, tag="hT", name="hT")
        w1a = {}

        def w1a_hook(t, zt):
            if t == 0:
                w1a["p"] = [ps.tile([128, ncr], F32, tag="pbig", name=f"ph_{m}")
                            for m in range(4)]
            if t % 16 == 0:
                kc = t // 16
                w1a["ch"] = [ws.tile([128, 16, 256], BF, tag="wsmall", bufs=3,
                                     name=f"w1a{mp}{kc}") for mp in range(2)]
                for mp in range(2):
                    nc.sync.dma_start(w1a["ch"][mp][:], w1t[mp * 2 + kc])
            for mp in range(2):
                for ml in range(2):
                    nc.tensor.matmul(w1a["p"][mp * 2 + ml][:],
                                     w1a["ch"][mp][:, t % 16, ml * 128:(ml + 1) * 128],
                                     zt, start=(t == 0), stop=(t == NT - 1))

        z3, x2_bf, _, _, _, _, rb2, nf2 = ln(
            None, ar3, ncr, "actD", "z3_bf", mode="separate", sep_tag="actA",
            fused_base=(z2, rb16_3, nb16_3), want_fix=True, hook=w1a_hook)

        def w1_fix(m, psrc):
            f1 = tp.tile([128, ncr], F32, tag="fixt", bufs=2, name=f"f1h{m}")
            nc.vector.tensor_mul(f1[:], psrc[:], rb2[:])
            f2 = tp.tile([128, ncr], F32, tag="fixt", bufs=2, name=f"f2h{m}")
            nc.vector.tensor_scalar(
                out=f2[:], in0=nf2[:], scalar1=sw1_sb[:, m:m + 1],
                scalar2=None, op0=mybir.AluOpType.mult)
            nc.vector.tensor_add(f1[:], f1[:], f2[:])
            nc.scalar.activation(hT[:, m, :], f1[:], AF.Gelu)

        for m in range(4):
            w1_fix(m, w1a["p"][m])
        # wave B (hid tiles 4..7) on the completed z3
        for mp in (2, 3):
            plsb = [ps.tile([128, ncr], F32, tag="pbig", name=f"phb{mp}_{m}")
                    for m in range(2)]
            for kc in range(2):
                ch = ws.tile([128, 16, 256], BF, tag="wsmall", bufs=3,
                             name=f"w1b{mp}{kc}")
                nc.sync.dma_start(ch[:], w1t[mp * 2 + kc])
                for t16 in range(16):
                    t = kc * 16 + t16
                    for ml in range(2):
                        nc.tensor.matmul(plsb[ml][:], ch[:, t16, ml * 128:(ml + 1) * 128],
                                         z3[:, t, :], start=(t == 0), stop=(t == 31))
            for ml in range(2):
                w1_fix(mp * 2 + ml, plsb[ml])
        ar4ins, ar4outs = ar_pair(ncr, 4, "f")
        for ci in range(8):
            ch = ws.tile([128, 8, 512], BF, tag="wsmall", bufs=3, name=f"w2c{ci}")
            nc.sync.dma_start(ch[:], w2t[ci])
            for tl in range(4):
                t = ci * 4 + tl
                pps = ps.tile([128, ncr], F32, tag="pbig", name=f"pw2{t}")
                for th in range(8):
                    nc.tensor.matmul(pps[:], ch[:, th, tl * 128:(tl + 1) * 128],
                                     hT[:, th, :], start=(th == 0), stop=(th == 7))
                stage_and_reduce(t, ncr, pps, ar4ins, ar4outs, "f")

        # ---- LN4 stats + scorer matmul on pre-norm z (normalized on host)
        sp_sb = sb.tile([128, NT, 1], BF, tag="spt", name="sp_sb")
        nc.sync.dma_start(sp_sb[:], spt[:])
        lps = ps.tile([1, ncr], F32, tag="pbig", name="lps")

        def spz_mm(t, zt):
            nc.tensor.matmul(lps[:], sp_sb[:, t, :], zt,
                             start=(t == 0), stop=(t == NT - 1))

        _, _, r4, nm4, _, _, _, _ = ln(x2_bf, ar4outs, ncr, "", "z4",
                                       mode="none", extra_mm=spz_mm)
        lg = tp.tile([1, ncr], F32, tag="lns", bufs=4, name="lg")
        nc.vector.tensor_copy(lg[:], lps[:])
        nc.sync.dma_start(spz_d[:], lg[:])
        nc.sync.dma_start(r4_d[:], r4[:])
        nc.sync.dma_start(nm4_d[:], nm4[:])

    nc.compile()
    return nc


# ---------------------------------------------------------------- entry point
def kernel(**inputs):
    global LAST_EXEC_NS
    vf = np.asarray(inputs["vision_feature"], np.float32)
    te = np.asarray(inputs["text_embed"], np.float32)
    mask = np.asarray(inputs["attention_mask"])

    thr, uniq, remained = _route_np(vf, te, mask)
    cat = np.concatenate([vf[uniq], te], 0)
    rem = vf[remained]
    ncu, ncr = cat.shape[0], rem.shape[0]
    lc = -(-ncu // 128) * 128
    lr = -(-ncr // 128) * 128

    key = (lc, lr, ncu, ncr)
    if key not in _CACHE:
        _CACHE[key] = _build(*key)
    nc = _CACHE[key]

    catT = _pad_t(cat.astype(BF16), lc)
    remT = _pad_t(rem.astype(BF16), lr)

    def _eb(nvalid, lpad):
        v = nvalid - (lpad // 128 - 1) * 128
        b = np.zeros((128, 1), np.float32)
        b[v:] = -1e5
        return b

    eb_cat = _eb(ncu, lc)
    eb_rem = _eb(ncr, lr)

    in_maps = []
    for c in range(NCORES):
        hs = slice(c * DHC, (c + 1) * DHC)
        m = {"catT": catT, "remT": remT, "eb_cat": eb_cat, "eb_rem": eb_rem,
             "spt": _shuffle(np.ascontiguousarray(
                 np.asarray(inputs["sp_w"], np.float32).T.reshape(D, 1).astype(BF16)))}
        for i, w in enumerate(("sa1_w", "sa2_w", "ca_w")):
            win = np.asarray(inputs[w], np.float32)
            wq, wk, wv = win[:D][hs], win[D:2 * D][hs], win[2 * D:][hs]
            sh = _shuffle(np.ascontiguousarray(
                np.concatenate([wq.T, wk.T, wv.T], 1)).astype(BF16))
            if w == "ca_w":
                m["wqkv2"] = np.stack([
                    sh[:, kc * 4:(kc + 1) * 4, grp * 512:(grp + 1) * 512]
                    for grp in range(3) for kc in range(8)])
                m["sq3"] = _colsum_tile(wq.astype(BF16))
            else:
                m[f"wqkv{i}"] = np.stack([
                    sh[:, kc * 4:(kc + 1) * 4, grp * 768:(grp + 1) * 768]
                    for grp in range(2) for kc in range(8)])
        for i, w in enumerate(("sa1_ow", "sa2_ow", "ca_ow")):
            wout = np.asarray(inputs[w], np.float32)
            sh = _shuffle(np.ascontiguousarray(wout[:, hs].T).astype(BF16))
            m[f"wo{i}"] = np.stack([sh[:, :, ci * 512:(ci + 1) * 512]
                                    for ci in range(8)])
        w1c = np.asarray(inputs["ffn_w1"], np.float32)[c * HIDC:(c + 1) * HIDC]
        m["sw1"] = _colsum_tile(w1c.astype(BF16))
        sh = _shuffle(np.ascontiguousarray(w1c.T).astype(BF16))
        m["w1t"] = np.stack([sh[:, kc * 16:(kc + 1) * 16, mp * 256:(mp + 1) * 256]
                             for mp in range(4) for kc in range(2)])
        sh = _shuffle(np.ascontiguousarray(
            np.asarray(inputs["ffn_w2"], np.float32)[:, c * HIDC:(c + 1) * HIDC].T
        ).astype(BF16))
        m["w2t"] = np.stack([sh[:, :, ci * 512:(ci + 1) * 512] for ci in range(8)])
        in_maps.append(m)

    from concourse import bass_utils
    res = bass_utils.run_bass_kernel_spmd(nc, in_maps, core_ids=list(range(NCORES)))
    LAST_EXEC_NS = res.exec_time_ns

    rr = res.results[0]
    spz = np.asarray(rr["spz"], np.float32)[0]
    r4 = np.asarray(rr["r4"], np.float32)[0]
    nm4 = np.asarray(rr["nm4"], np.float32)[0]
    s_sp = np.float32(np.asarray(inputs["sp_w"], np.float32)
                      .astype(BF16).astype(np.float32).sum())
    logit = r4 * spz + s_sp * nm4 + np.float32(inputs["sp_b"][0])
    k = max(int(thr * EXPAND_RATIO), 1)
    gi = np.argsort(-logit, kind="stable")[:k]
    final = np.unique(np.concatenate([uniq, remained[gi]]))
    return vf[final]


# revision 29
# speedup vs baseline: 1.0687x; 1.0122x over previous
"""Trainium2 Bass kernel for nn_CosSimRouter_learnable_pad.

Host: routing (tiny, exact fp32 replication of the reference) + final top-k /
gather. Device (8 NeuronCores, Megatron tensor-parallel): the ExpanderModule
(3 MHA blocks + FFN + 4 LayerNorms + scorer) with MHA heads and FFN hidden dim
sharded across cores, bf16 matmuls with fp32 accumulation, chunked bf16
AllReduce after each out-projection and after ffn_w2, pipelined with compute.

Key scheduling tricks: exact (unpadded) Q-side widths; LayerNorm deferred-apply
so MHA3's Q-projection and ffn_w1 run on pre-norm activations during the
AllReduce, corrected afterwards with the rank-1 LN fixup; final LN + scorer
folded into host math on (sp.z, r, -mu*r).

Self-contained: takes full inputs, returns the full output.
"""

import numpy as np
import ml_dtypes

BF16 = ml_dtypes.bfloat16

GRID = 24
HEADS = 16
D = 4096
HID = 8192
LV = GRID * GRID
LT = 64
GAMMA = 0.065
TEMP = 0.05
EXPAND_RATIO = 0.3
NCORES = 8
DH = D // HEADS            # 256 per head
NH_CORE = HEADS // NCORES  # 2 heads per core
DHC = DH * NH_CORE         # 512 per-core head dims
HIDC = HID // NCORES       # 1024 per-core ffn hidden
NT = D // 128              # 32 D-tiles

LAST_EXEC_NS = None
_CACHE = {}


# ---------------------------------------------------------------- host routing
def _route_np(vf, te, mask):
    """Exact fp32 replication of reference._route (numpy)."""
    vn = vf / np.maximum(np.linalg.norm(vf, axis=-1, keepdims=True), np.float32(1e-8))
    tn = te / np.maximum(np.linalg.norm(te, axis=-1, keepdims=True), np.float32(1e-8))
    cs = np.where(mask, (vn @ tn.T).astype(np.float32), np.float32(0.0))
    m = cs.max(-1) / np.float32(TEMP)
    e = np.exp(m - m.max())
    scores = e / e.sum()
    order = np.argsort(-scores, kind="stable")
    cum = np.cumsum(scores[order])
    thr = max(int((cum <= np.float32(GAMMA)).sum()), 1)
    selected = order[:thr]
    offs = np.array([[i, j] for i in (-1, 0, 1) for j in (-1, 0, 1)
                     if not (i == 0 and j == 0)])
    r = np.clip(selected[:, None] // GRID + offs[None, :, 0], 0, GRID - 1)
    c = np.clip(selected[:, None] % GRID + offs[None, :, 1], 0, GRID - 1)
    uniq = np.unique((r * GRID + c).reshape(-1))
    remained = np.setdiff1d(np.arange(LV), uniq)
    return thr, uniq, remained


def _shuffle(m):
    """[K, N] -> [128, K//128, N] so device tile [:, t, :] = rows t*128..t*128+128."""
    k, n = m.shape
    return np.ascontiguousarray(m.reshape(k // 128, 128, n).transpose(1, 0, 2))


def _pad_t(x, lp):
    """x [L, D] -> shuffled transpose [128, 32, lp] (zero-padded columns)."""
    out = np.zeros((D, lp), x.dtype)
    out[:, : x.shape[0]] = x.T
    return _shuffle(out)


def _colsum_tile(w):
    """w [F, D] bf16 -> [128, F//128] f32 column-sum tile ([p, m] = sum_d w[m*128+p])."""
    s = w.astype(np.float32).sum(1)
    return np.ascontiguousarray(s.reshape(-1, 128).T)


# ---------------------------------------------------------------- bass builder
def _build(lc, lr, ncu, ncr):
    from contextlib import ExitStack
    import concourse.bass as bass
    import concourse.tile as tile
    from concourse import bacc, mybir

    BF = mybir.dt.bfloat16
    F32 = mybir.dt.float32
    AF = mybir.ActivationFunctionType
    RG = [list(range(NCORES))]

    nc = bacc.Bacc("TRN2", target_bir_lowering=False, debug=False,
                   num_devices=NCORES)

    catT = nc.dram_tensor("catT", [128, NT, lc], BF, kind="ExternalInput").ap()
    remT = nc.dram_tensor("remT", [128, NT, lr], BF, kind="ExternalInput").ap()
    wqkv = [nc.dram_tensor(f"wqkv{i}", [16, 128, 4, 768], BF,
                           kind="ExternalInput").ap() for i in range(2)]
    wqkv3 = nc.dram_tensor("wqkv2", [24, 128, 4, 512], BF,
                           kind="ExternalInput").ap()
    wo = [nc.dram_tensor(f"wo{i}", [8, 128, 4, 512], BF,
                         kind="ExternalInput").ap() for i in range(3)]
    w1t = nc.dram_tensor("w1t", [8, 128, 16, 256], BF, kind="ExternalInput").ap()
    w2t = nc.dram_tensor("w2t", [8, 128, 8, 512], BF, kind="ExternalInput").ap()
    spt = nc.dram_tensor("spt", [128, NT, 1], BF, kind="ExternalInput").ap()
    eb_cat = nc.dram_tensor("eb_cat", [128, 1], F32, kind="ExternalInput").ap()
    eb_rem = nc.dram_tensor("eb_rem", [128, 1], F32, kind="ExternalInput").ap()
    sq3_d = nc.dram_tensor("sq3", [128, 4], F32, kind="ExternalInput").ap()
    sw1_d = nc.dram_tensor("sw1", [128, 8], F32, kind="ExternalInput").ap()
    spz_d = nc.dram_tensor("spz", [1, ncr], F32, kind="ExternalOutput").ap()
    r4_d = nc.dram_tensor("r4", [1, ncr], F32, kind="ExternalOutput").ap()
    nm4_d = nc.dram_tensor("nm4", [1, ncr], F32, kind="ExternalOutput").ap()

    with tile.TileContext(nc) as tc, ExitStack() as ctx:
        sb = ctx.enter_context(tc.tile_pool(name="sb", bufs=1))
        ws = ctx.enter_context(tc.tile_pool(name="ws", bufs=3))
        tp = ctx.enter_context(tc.tile_pool(name="tp", bufs=2))
        ps = ctx.enter_context(tc.tile_pool(name="ps", bufs=6, space="PSUM"))
        pst = ctx.enter_context(tc.tile_pool(name="pst", bufs=2, space="PSUM"))
        dr = ctx.enter_context(tc.tile_pool(name="dr", bufs=1, space="DRAM"))

        ones_bf = sb.tile([128, 1], BF, tag="ones", name="ones_bf")
        nc.vector.memset(ones_bf[:], 1.0)
        ones_row = sb.tile([1, 128], F32, tag="onesr", name="ones_row")
        nc.vector.memset(ones_row[:], 1.0)
        eps_t = sb.tile([1, 1], F32, tag="eps", name="eps_t")
        nc.vector.memset(eps_t[:], 1e-5)

        cat_sb = sb.tile([128, NT, lc], BF, tag="actC", name="cat_sb")
        nc.sync.dma_start(cat_sb[:], catT[:])
        rem_sb = sb.tile([128, NT, lr], BF, tag="actA", name="rem_sb")
        ebc_sb = sb.tile([128, 1], F32, tag="ebc", name="ebc_sb")
        nc.sync.dma_start(ebc_sb[:], eb_cat[:])
        ebr_sb = sb.tile([128, 1], F32, tag="ebr", name="ebr_sb")
        nc.sync.dma_start(ebr_sb[:], eb_rem[:])
        sq3_sb = sb.tile([128, 4], F32, tag="sq3", name="sq3_sb")
        nc.sync.dma_start(sq3_sb[:], sq3_d[:])
        sw1_sb = sb.tile([128, 8], F32, tag="sw1", name="sw1_sb")
        nc.sync.dma_start(sw1_sb[:], sw1_d[:])

        def bcast(row_f32, lq, nm):
            """[1, lq] f32 -> psum [128, lq] f32 via K=1 outer-product matmul."""
            pb = ps.tile([128, lq], F32, tag="pbig", name=f"bc{nm}")
            nc.tensor.matmul(pb[:], ones_row[:], row_f32, start=True, stop=True)
            return pb

        def ar_pair(lq, nch, nm):
            tpc = NT // nch
            ins_ = [dr.tile([128, tpc, lq], BF, tag=f"ai{nm}{g}", name=f"ai{nm}{g}")
                    for g in range(nch)]
            outs_ = [dr.tile([128, tpc, lq], BF, tag=f"ao{nm}{g}", name=f"ao{nm}{g}")
                     for g in range(nch)]
            return ins_, outs_

        def stage_and_reduce(t, lq, pps, arins, arouts, nm):
            """Copy psum tile t into the staging buffer; every 4 tiles DMA to the
            AR chunk buffer; when a chunk completes, launch its AllReduce."""
            tpc = NT // len(arins)
            g, t4 = t // 4, t % 4
            if t4 == 0:
                stage_and_reduce.cur = tp.tile([128, 4, lq], BF, tag="abig",
                                               bufs=2, name=f"ab{nm}{g}")
            nc.scalar.copy(stage_and_reduce.cur[:, t4, :], pps[:])
            if t4 == 3:
                c = t // tpc
                off = (g % (tpc // 4)) * 4
                nc.sync.dma_start(arins[c][:, off:off + 4, :],
                                  stage_and_reduce.cur[:])
                if t == (c + 1) * tpc - 1:
                    nc.gpsimd.collective_compute(
                        "AllReduce", mybir.AluOpType.add, replica_groups=RG,
                        ins=[arins[c].opt()], outs=[arouts[c].opt()])

        def attention_and_outproj(widx, qT, kT, vv, lq, lkp, eb_sb, wo_d):
            nlk = lkp // 128
            # ---- attention per head (softmax without max-subtraction)
            oT = tp.tile([128, 4, lq], BF, tag="oT", bufs=1, name=f"oT{widx}")
            for h in range(NH_CORE):
                expT = tp.tile([128, nlk, lq], BF, tag="expT", bufs=1,
                               name=f"expT{widx}_{h}")
                for lkt in range(nlk):
                    sps = ps.tile([128, lq], F32, tag="pbig", name=f"psc{widx}{h}{lkt}")
                    for td in range(2):
                        nc.tensor.matmul(sps[:],
                                         kT[:, h * 2 + td, lkt * 128:(lkt + 1) * 128],
                                         qT[:, h * 2 + td, :],
                                         start=(td == 0), stop=(td == 1))
                    bias = eb_sb[:] if lkt == nlk - 1 else 0.0
                    nc.scalar.activation(expT[:, lkt, :], sps[:], AF.Exp,
                                         scale=1.0 / 16.0, bias=bias)
                dps = pst.tile([1, lq], F32, tag="pstat", name=f"pd{widx}{h}")
                for lkt in range(nlk):
                    nc.tensor.matmul(dps[:], ones_bf[:], expT[:, lkt, :],
                                     start=(lkt == 0), stop=(lkt == nlk - 1))
                rc = tp.tile([1, lq], F32, tag="recip", bufs=1, name=f"rc{widx}{h}")
                nc.vector.reciprocal(rc[:], dps[:])
                rbp = bcast(rc[:], lq, f"r{widx}{h}")
                rbs = tp.tile([128, lq], F32, tag="rbs", bufs=1, name=f"rbs{widx}{h}")
                nc.scalar.copy(rbs[:], rbp[:])
                for td in range(2):
                    ops_ = ps.tile([128, lq], F32, tag="pbig", name=f"po{widx}{h}{td}")
                    for lkt in range(nlk):
                        nc.tensor.matmul(ops_[:],
                                         vv[:, lkt, h * 256 + td * 128:h * 256 + (td + 1) * 128],
                                         expT[:, lkt, :],
                                         start=(lkt == 0), stop=(lkt == nlk - 1))
                    nc.vector.tensor_mul(oT[:, h * 2 + td, :], ops_[:], rbs[:])
            # ---- out projection (row-parallel) + chunked AllReduce
            arins, arouts = ar_pair(lq, {0: 1, 1: 2, 2: 2}[widx], f"m{widx}")
            for ci in range(8):
                ch = ws.tile([128, 4, 512], BF, tag="wsmall", bufs=3, name=f"woc{widx}{ci}")
                nc.sync.dma_start(ch[:], wo_d[ci])
                for tl in range(4):
                    t = ci * 4 + tl
                    pps = ps.tile([128, lq], F32, tag="pbig", name=f"pop{widx}{t}")
                    for td in range(4):
                        nc.tensor.matmul(pps[:], ch[:, td, tl * 128:(tl + 1) * 128],
                                         oT[:, td, :], start=(td == 0), stop=(td == 3))
                    stage_and_reduce(t, lq, pps, arins, arouts, f"m{widx}")
            return arouts

        def mha(widx, xq, lq, xkv, lkp, eb_sb):
            """One TP-sharded MHA block; returns chunked AllReduce output tiles.

            xq: [128, NT, >=lq] tile (q-side rhs sliced to exact lq).
            xkv: [128, NT, lkp] tile (k/v side, lkp padded to x128, eb masks pad).
            """
            nlk = lkp // 128
            qT = tp.tile([128, 4, lq], BF, tag="qT", bufs=1, name=f"qT{widx}")
            kT = tp.tile([128, 4, lkp], BF, tag="kT", bufs=1, name=f"kT{widx}")
            vv = tp.tile([128, nlk, DHC], BF, tag="vv", bufs=1, name=f"vv{widx}")
            # ---- fused QKV projection, weight-streamed in two column groups
            for grp in (0, 1):
                if grp == 0:  # cols 0:768 -> q0..q3, k0, k1
                    pls = [ps.tile([128, lq], F32, tag="pbig",
                                   name=f"pq{widx}_{m}") for m in range(4)]
                    pls += [ps.tile([128, lkp], F32, tag="pbig",
                                    name=f"pk{widx}_{m}") for m in range(2)]
                else:  # cols 768:1536 -> k2, k3, v rows
                    pls = [ps.tile([128, lkp], F32, tag="pbig",
                                   name=f"pk{widx}_{2 + m}") for m in range(2)]
                    pls += [ps.tile([128, DHC], F32, tag="pbig",
                                    name=f"pv{widx}_{m}") for m in range(nlk)]
                for kc in range(8):
                    ch = ws.tile([128, 4, 768], BF, tag="wqkvch", bufs=4, name=f"wc{widx}{grp}{kc}")
                    nc.sync.dma_start(ch[:], wqkv[widx][grp * 8 + kc])
                    for t8 in range(4):
                        t = kc * 4 + t8
                        st, sp_ = (t == 0), (t == 31)
                        if grp == 0:
                            for m in range(4):
                                nc.tensor.matmul(pls[m][:], ch[:, t8, m * 128:(m + 1) * 128],
                                                 xq[:, t, 0:lq], start=st, stop=sp_)
                            for m in range(2):
                                nc.tensor.matmul(pls[4 + m][:],
                                                 ch[:, t8, 512 + m * 128:512 + (m + 1) * 128],
                                                 xkv[:, t, :], start=st, stop=sp_)
                        else:
                            for m in range(2):
                                nc.tensor.matmul(pls[m][:], ch[:, t8, m * 128:(m + 1) * 128],
                                                 xkv[:, t, :], start=st, stop=sp_)
                            for mk in range(nlk):
                                nc.tensor.matmul(pls[2 + mk][:],
                                                 xkv[:, t, mk * 128:(mk + 1) * 128],
                                                 ch[:, t8, 256:768], start=st, stop=sp_)
                if grp == 0:
                    for m in range(4):
                        nc.scalar.copy(qT[:, m, :], pls[m][:])
                    for m in range(2):
                        nc.scalar.copy(kT[:, m, :], pls[4 + m][:])
                else:
                    for m in range(2):
                        nc.scalar.copy(kT[:, 2 + m, :], pls[m][:])
                    for mk in range(nlk):
                        nc.scalar.copy(vv[:, mk, :], pls[2 + mk][:])
            arouts = attention_and_outproj(widx, qT, kT, vv, lq, lkp, eb_sb,
                                            wo[widx])
            return arouts

        def ln(base, arouts, lq, out_tag, out_name, lpad=None, mode="inplace",
               fused_base=None, extra_mm=None, want_fix=False, sep_tag=None,
               hook=None):
            """z = base + ar (optionally base = z_pre*rb + nb fused from a
            deferred LN); stats accumulate per arriving AllReduce chunk.
            mode: "inplace" (normalize z in place), "separate" (keep z pre-norm,
            write normalized copy to sep_tag tile), "defer" (keep z pre-norm,
            return bf16+f32 row broadcasts for downstream fixup/fusion),
            "none" (z transient, stats only).
            Returns (z, applied, r, nmr, rb16, nb16, rb_s, nmrb_s)."""
            z = None
            if mode != "none":
                zw = lpad if lpad is not None else lq
                z = sb.tile([128, NT, zw], BF, tag=out_tag, name=out_name)
                if zw > lq:
                    nc.vector.memset(z[:, :, lq:zw], 0.0)
            sums = pst.tile([1, lq], F32, tag="pstat", name=f"su{out_name}")
            sqs = pst.tile([1, lq], F32, tag="pstat", name=f"sq{out_name}")
            tpc = NT // len(arouts)
            for g in range(NT // 4):
                arB = tp.tile([128, 4, lq], BF, tag="arB", bufs=2,
                              name=f"arB{out_name}{g}")
                c = (g * 4) // tpc
                off = (g * 4) % tpc
                nc.sync.dma_start(arB[:], arouts[c][:, off:off + 4, :])
                for t4 in range(4):
                    t = g * 4 + t4
                    if mode != "none":
                        zt = z[:, t, 0:lq]
                    else:
                        ztile = tp.tile([128, lq], BF, tag="z4t", bufs=2,
                                        name=f"zt{out_name}{t}")
                        zt = ztile[:]
                    if fused_base is not None:
                        zp, frb, fnb = fused_base
                        fz = tp.tile([128, lq], BF, tag="lnt", bufs=2,
                                     name=f"fz{out_name}{t}")
                        nc.vector.tensor_mul(fz[:], zp[:, t, 0:lq], frb[:])
                        nc.vector.tensor_add(fz[:], fz[:], fnb[:])
                        nc.vector.tensor_add(zt, fz[:], arB[:, t4, :])
                    else:
                        nc.vector.tensor_add(zt, base[:, t, 0:lq], arB[:, t4, :])
                    nc.tensor.matmul(sums[:], ones_bf[:], zt,
                                     start=(t == 0), stop=(t == NT - 1))
                    sq = tp.tile([128, lq], BF, tag="sq", bufs=2,
                                 name=f"q{out_name}{t}")
                    nc.vector.tensor_mul(sq[:], zt, zt)
                    nc.tensor.matmul(sqs[:], ones_bf[:], sq[:],
                                     start=(t == 0), stop=(t == NT - 1))
                    if extra_mm is not None:
                        extra_mm(t, zt)
                    if hook is not None:
                        hook(t, zt)
            mu = tp.tile([1, lq], F32, tag="lns", bufs=4, name=f"mu{out_name}")
            nc.scalar.mul(mu[:], sums[:], 1.0 / D)
            ex2 = tp.tile([1, lq], F32, tag="lns", bufs=4, name=f"e2{out_name}")
            nc.scalar.mul(ex2[:], sqs[:], 1.0 / D)
            tmp = tp.tile([1, lq], F32, tag="lns", bufs=4, name=f"va{out_name}")
            nc.vector.tensor_mul(tmp[:], mu[:], mu[:])
            nc.vector.tensor_sub(tmp[:], ex2[:], tmp[:])
            nc.scalar.activation(tmp[:], tmp[:], AF.Sqrt, bias=eps_t[:])
            r_ = tp.tile([1, lq], F32, tag="lns", bufs=4, name=f"r{out_name}")
            nc.vector.reciprocal(r_[:], tmp[:])
            nmr = mu
            nc.vector.tensor_mul(nmr[:], nmr[:], r_[:])
            nc.scalar.mul(nmr[:], nmr[:], -1.0)
            if mode == "none":
                return None, None, r_, nmr, None, None, None, None
            rbp = bcast(r_[:], lq, f"lr{out_name}")
            rb16 = tp.tile([128, lq], BF, tag="lnb", bufs=2, name=f"rb{out_name}")
            nc.scalar.copy(rb16[:], rbp[:])
            rb_s = None
            if want_fix:
                rb_s = tp.tile([128, lq], F32, tag="lnbf", bufs=2,
                               name=f"rf{out_name}")
                nc.scalar.copy(rb_s[:], rbp[:])
            nbp = bcast(nmr[:], lq, f"ln{out_name}")
            nb16 = tp.tile([128, lq], BF, tag="lnb", bufs=2, name=f"nb{out_name}")
            nc.scalar.copy(nb16[:], nbp[:])
            nmrb_s = None
            if want_fix:
                nmrb_s = tp.tile([128, lq], F32, tag="lnbf", bufs=2,
                                 name=f"nf{out_name}")
                nc.scalar.copy(nmrb_s[:], nbp[:])
            applied = None
            if mode == "inplace":
                for t in range(NT):
                    tm = tp.tile([128, lq], BF, tag="lnt", bufs=2,
                                 name=f"tm{out_name}{t}")
                    nc.vector.tensor_mul(tm[:], z[:, t, 0:lq], rb16[:])
                    nc.vector.tensor_add(z[:, t, 0:lq], tm[:], nb16[:])
                applied = z
            elif mode == "separate":
                applied = sb.tile([128, NT, lq], BF, tag=sep_tag,
                                  name=f"{out_name}_ap")
                for t in range(NT):
                    tm = tp.tile([128, lq], BF, tag="lnt", bufs=2,
                                 name=f"tm{out_name}{t}")
                    nc.vector.tensor_mul(tm[:], z[:, t, 0:lq], rb16[:])
                    nc.vector.tensor_add(applied[:, t, :], tm[:], nb16[:])
            return z, applied, r_, nmr, rb16, nb16, rb_s, nmrb_s

        # branch A (cat) and branch B (rem) are independent up to MHA3
        ar1 = mha(0, cat_sb, ncu, cat_sb, lc, ebc_sb)
        nc.sync.dma_start(rem_sb[:], remT[:])
        ar2 = mha(1, rem_sb, ncr, rem_sb, lr, ebr_sb)
        # x: applied LN1 output, padded to lc for use as MHA3's k/v side
        x_bf, _, _, _, _, _, _, _ = ln(cat_sb, ar1, ncu, "actD", "x_bf",
                                       lpad=lc, mode="inplace")
        x_bf = x_bf  # applied in place

        # ---- MHA3 K/V projection on x (early, independent of AR2)
        nlk3 = lc // 128
        kT3 = tp.tile([128, 4, lc], BF, tag="kT", bufs=1, name="kT3")
        vv3 = tp.tile([128, nlk3, DHC], BF, tag="vv", bufs=1, name="vv3")
        for grp in (1, 2):
            if grp == 1:
                pls3 = [ps.tile([128, lc], F32, tag="pbig", name=f"pk2_{m}")
                        for m in range(4)]
            else:
                pls3 = [ps.tile([128, DHC], F32, tag="pbig", name=f"pv2_{m}")
                        for m in range(nlk3)]
            for kc in range(8):
                ch = ws.tile([128, 4, 512], BF, tag="wqkvch", bufs=4,
                             name=f"wc3{grp}{kc}")
                nc.sync.dma_start(ch[:], wqkv3[grp * 8 + kc])
                for t8 in range(4):
                    t = kc * 4 + t8
                    st, sp_ = (t == 0), (t == 31)
                    if grp == 1:
                        for m in range(4):
                            nc.tensor.matmul(pls3[m][:], ch[:, t8, m * 128:(m + 1) * 128],
                                             x_bf[:, t, :], start=st, stop=sp_)
                    else:
                        for mk in range(nlk3):
                            nc.tensor.matmul(pls3[mk][:],
                                             x_bf[:, t, mk * 128:(mk + 1) * 128],
                                             ch[:, t8, :], start=st, stop=sp_)
            if grp == 1:
                for m in range(4):
                    nc.scalar.copy(kT3[:, m, :], pls3[m][:])
            else:
                for mk in range(nlk3):
                    nc.scalar.copy(vv3[:, mk, :], pls3[mk][:])

        # ---- LN3 (deferred) with MHA3's Q-projection fused into the chunk loop
        q3 = {}

        def q3_hook(t, zt):
            if t == 0:
                q3["p"] = [ps.tile([128, ncr], F32, tag="pbig", name=f"pq2_{m}")
                           for m in range(4)]
            if t % 4 == 0:
                q3["ch"] = ws.tile([128, 4, 512], BF, tag="wqkvch", bufs=4,
                                   name=f"wcq3{t // 4}")
                nc.sync.dma_start(q3["ch"][:], wqkv3[t // 4])
            for m in range(4):
                nc.tensor.matmul(q3["p"][m][:],
                                 q3["ch"][:, t % 4, m * 128:(m + 1) * 128],
                                 zt, start=(t == 0), stop=(t == NT - 1))

        z2, _, _, _, rb16_3, nb16_3, rb3, nf3 = ln(
            rem_sb, ar2, ncr, "actB", "z2_bf", mode="defer", want_fix=True,
            hook=q3_hook)
        qT3 = tp.tile([128, 4, ncr], BF, tag="qT", bufs=1, name="qT3")
        for m in range(4):
            f1 = tp.tile([128, ncr], F32, tag="fixt", bufs=2, name=f"f1q3{m}")
            nc.vector.tensor_mul(f1[:], q3["p"][m][:], rb3[:])
            f2 = tp.tile([128, ncr], F32, tag="fixt", bufs=2, name=f"f2q3{m}")
            nc.vector.tensor_scalar(
                out=f2[:], in0=nf3[:], scalar1=sq3_sb[:, m:m + 1],
                scalar2=None, op0=mybir.AluOpType.mult)
            nc.vector.tensor_add(qT3[:, m, :], f1[:], f2[:])
        ar3 = attention_and_outproj(2, qT3, kT3, vv3, ncr, lc, ebc_sb, wo[2])

        # ---- LN2 with FFN w1 wave-A (hid tiles 0..3) fused into the chunk loop
        hT = sb.tile([128, HIDC // 128, ncr], BF, tag="hT", name="hT")
        w1a = {}

        def w1a_hook(t, zt):
            if t == 0:
                w1a["p"] = [ps.tile([128, ncr], F32, tag="pbig", name=f"ph_{m}")
                            for m in range(4)]
            if t % 16 == 0:
                kc = t // 16
                w1a["ch"] = [ws.tile([128, 16, 256], BF, tag="wsmall", bufs=3,
                                     name=f"w1a{mp}{kc}") for mp in range(2)]
                for mp in range(2):
                    nc.sync.dma_start(w1a["ch"][mp][:], w1t[mp * 2 + kc])
            for mp in range(2):
                for ml in range(2):
                    nc.tensor.matmul(w1a["p"][mp * 2 + ml][:],
                                     w1a["ch"][mp][:, t % 16, ml * 128:(ml + 1) * 128],
                                     zt, start=(t == 0), stop=(t == NT - 1))

        z3, x2_bf, _, _, _, _, rb2, nf2 = ln(
            None, ar3, ncr, "actD", "z3_bf", mode="separate", sep_tag="actA",
            fused_base=(z2, rb16_3, nb16_3), want_fix=True, hook=w1a_hook)

        def w1_fix(m, psrc):
            f1 = tp.tile([128, ncr], F32, tag="fixt", bufs=2, name=f"f1h{m}")
            nc.vector.tensor_mul(f1[:], psrc[:], rb2[:])
            f2 = tp.tile([128, ncr], F32, tag="fixt", bufs=2, name=f"f2h{m}")
            nc.vector.tensor_scalar(
                out=f2[:], in0=nf2[:], scalar1=sw1_sb[:, m:m + 1],
                scalar2=None, op0=mybir.AluOpType.mult)
            nc.vector.tensor_add(f1[:], f1[:], f2[:])
            nc.scalar.activation(hT[:, m, :], f1[:], AF.Gelu)

        for m in range(4):
            w1_fix(m, w1a["p"][m])
        # wave B (hid tiles 4..7) on the completed z3
        for mp in (2, 3):
            plsb = [ps.tile([128, ncr], F32, tag="pbig", name=f"phb{mp}_{m}")
                    for m in range(2)]
            for kc in range(2):
                ch = ws.tile([128, 16, 256], BF, tag="wsmall", bufs=3,
                             name=f"w1b{mp}{kc}")
                nc.sync.dma_start(ch[:], w1t[mp * 2 + kc])
                for t16 in range(16):
                    t = kc * 16 + t16
                    for ml in range(2):
                        nc.tensor.matmul(plsb[ml][:], ch[:, t16, ml * 128:(ml + 1) * 128],
                                         z3[:, t, :], start=(t == 0), stop=(t == 31))
            for ml in range(2):
                w1_fix(mp * 2 + ml, plsb[ml])
        ar4ins, ar4outs = ar_pair(ncr, 2, "f")
        for ci in range(8):
            ch = ws.tile([128, 8, 512], BF, tag="wsmall", bufs=3, name=f"w2c{ci}")
            nc.sync.dma_start(ch[:], w2t[ci])
            for tl in range(4):
                t = ci * 4 + tl
                pps = ps.tile([128, ncr], F32, tag="pbig", name=f"pw2{t}")
                for th in range(8):
                    nc.tensor.matmul(pps[:], ch[:, th, tl * 128:(tl + 1) * 128],
                                     hT[:, th, :], start=(th == 0), stop=(th == 7))
                stage_and_reduce(t, ncr, pps, ar4ins, ar4outs, "f")

        # ---- LN4 stats + scorer matmul on pre-norm z (normalized on host)
        sp_sb = sb.tile([128, NT, 1], BF, tag="spt", name="sp_sb")
        nc.sync.dma_start(sp_sb[:], spt[:])
        lps = ps.tile([1, ncr], F32, tag="pbig", name="lps")

        def spz_mm(t, zt):
            nc.tensor.matmul(lps[:], sp_sb[:, t, :], zt,
                             start=(t == 0), stop=(t == NT - 1))

        _, _, r4, nm4, _, _, _, _ = ln(x2_bf, ar4outs, ncr, "", "z4",
                                       mode="none", extra_mm=spz_mm)
        lg = tp.tile([1, ncr], F32, tag="lns", bufs=4, name="lg")
        nc.vector.tensor_copy(lg[:], lps[:])
        nc.sync.dma_start(spz_d[:], lg[:])
        nc.sync.dma_start(r4_d[:], r4[:])
        nc.sync.dma_start(nm4_d[:], nm4[:])

    nc.compile()
    return nc


# ---------------------------------------------------------------- entry point
def kernel(**inputs):
    global LAST_EXEC_NS
    vf = np.asarray(inputs["vision_feature"], np.float32)
    te = np.asarray(inputs["text_embed"], np.float32)
    mask = np.asarray(inputs["attention_mask"])

    thr, uniq, remained = _route_np(vf, te, mask)
    cat = np.concatenate([vf[uniq], te], 0)
    rem = vf[remained]
    ncu, ncr = cat.shape[0], rem.shape[0]
    lc = -(-ncu // 128) * 128
    lr = -(-ncr // 128) * 128

    key = (lc, lr, ncu, ncr)
    if key not in _CACHE:
        _CACHE[key] = _build(*key)
    nc = _CACHE[key]

    catT = _pad_t(cat.astype(BF16), lc)
    remT = _pad_t(rem.astype(BF16), lr)

    def _eb(nvalid, lpad):
        v = nvalid - (lpad // 128 - 1) * 128
        b = np.zeros((128, 1), np.float32)
        b[v:] = -1e5
        return b

    eb_cat = _eb(ncu, lc)
    eb_rem = _eb(ncr, lr)

    in_maps = []
    for c in range(NCORES):
        hs = slice(c * DHC, (c + 1) * DHC)
        m = {"catT": catT, "remT": remT, "eb_cat": eb_cat, "eb_rem": eb_rem,
             "spt": _shuffle(np.ascontiguousarray(
                 np.asarray(inputs["sp_w"], np.float32).T.reshape(D, 1).astype(BF16)))}
        for i, w in enumerate(("sa1_w", "sa2_w", "ca_w")):
            win = np.asarray(inputs[w], np.float32)
            wq, wk, wv = win[:D][hs], win[D:2 * D][hs], win[2 * D:][hs]
            sh = _shuffle(np.ascontiguousarray(
                np.concatenate([wq.T, wk.T, wv.T], 1)).astype(BF16))
            if w == "ca_w":
                m["wqkv2"] = np.stack([
                    sh[:, kc * 4:(kc + 1) * 4, grp * 512:(grp + 1) * 512]
                    for grp in range(3) for kc in range(8)])
                m["sq3"] = _colsum_tile(wq.astype(BF16))
            else:
                m[f"wqkv{i}"] = np.stack([
                    sh[:, kc * 4:(kc + 1) * 4, grp * 768:(grp + 1) * 768]
                    for grp in range(2) for kc in range(8)])
        for i, w in enumerate(("sa1_ow", "sa2_ow", "ca_ow")):
            wout = np.asarray(inputs[w], np.float32)
            sh = _shuffle(np.ascontiguousarray(wout[:, hs].T).astype(BF16))
            m[f"wo{i}"] = np.stack([sh[:, :, ci * 512:(ci + 1) * 512]
                                    for ci in range(8)])
        w1c = np.asarray(inputs["ffn_w1"], np.float32)[c * HIDC:(c + 1) * HIDC]
        m["sw1"] = _colsum_tile(w1c.astype(BF16))
        sh = _shuffle(np.ascontiguousarray(w1c.T).astype(BF16))
        m["w1t"] = np.stack([sh[:, kc * 16:(kc + 1) * 16, mp * 256:(mp + 1) * 256]
                             for mp in range(4) for kc in range(2)])
        sh = _shuffle(np.ascontiguousarray(
            np.asarray(inputs["ffn_w2"], np.float32)[:, c * HIDC:(c + 1) * HIDC].T
        ).astype(BF16))
        m["w2t"] = np.stack([sh[:, :, ci * 512:(ci + 1) * 512] for ci in range(8)])
        in_maps.append(m)

    from concourse import bass_utils
    res = bass_utils.run_bass_kernel_spmd(nc, in_maps, core_ids=list(range(NCORES)))
    LAST_EXEC_NS = res.exec_time_ns

    rr = res.results[0]
    spz = np.asarray(rr["spz"], np.float32)[0]
    r4 = np.asarray(rr["r4"], np.float32)[0]
    nm4 = np.asarray(rr["nm4"], np.float32)[0]
    s_sp = np.float32(np.asarray(inputs["sp_w"], np.float32)
                      .astype(BF16).astype(np.float32).sum())
    logit = r4 * spz + s_sp * nm4 + np.float32(inputs["sp_b"][0])
    k = max(int(thr * EXPAND_RATIO), 1)
    gi = np.argsort(-logit, kind="stable")[:k]
    final = np.unique(np.concatenate([uniq, remained[gi]]))
    return vf[final]


# revision 30
# speedup vs baseline: 1.1237x; 1.0514x over previous
"""Trainium2 Bass kernel for nn_CosSimRouter_learnable_pad.

Host: routing (tiny, exact fp32 replication of the reference) + final top-k /
gather. Device (8 NeuronCores, Megatron tensor-parallel): the ExpanderModule
(3 MHA blocks + FFN + 4 LayerNorms + scorer) with MHA heads and FFN hidden dim
sharded across cores, bf16 matmuls with fp32 accumulation, chunked bf16
AllReduce after each out-projection and after ffn_w2, pipelined with compute.

Key scheduling tricks: exact (unpadded) Q-side widths; LayerNorm deferred-apply
so MHA3's Q-projection and ffn_w1 run on pre-norm activations during the
AllReduce, corrected afterwards with the rank-1 LN fixup; final LN + scorer
folded into host math on (sp.z, r, -mu*r).

Self-contained: takes full inputs, returns the full output.
"""

import numpy as np
import ml_dtypes

BF16 = ml_dtypes.bfloat16

GRID = 24
HEADS = 16
D = 4096
HID = 8192
LV = GRID * GRID
LT = 64
GAMMA = 0.065
TEMP = 0.05
EXPAND_RATIO = 0.3
NCORES = 8
DH = D // HEADS            # 256 per head
NH_CORE = HEADS // NCORES  # 2 heads per core
DHC = DH * NH_CORE         # 512 per-core head dims
HIDC = HID // NCORES       # 1024 per-core ffn hidden
NT = D // 128              # 32 D-tiles

LAST_EXEC_NS = None
_CACHE = {}


# ---------------------------------------------------------------- host routing
def _route_np(vf, te, mask):
    """Exact fp32 replication of reference._route (numpy)."""
    vn = vf / np.maximum(np.linalg.norm(vf, axis=-1, keepdims=True), np.float32(1e-8))
    tn = te / np.maximum(np.linalg.norm(te, axis=-1, keepdims=True), np.float32(1e-8))
    cs = np.where(mask, (vn @ tn.T).astype(np.float32), np.float32(0.0))
    m = cs.max(-1) / np.float32(TEMP)
    e = np.exp(m - m.max())
    scores = e / e.sum()
    order = np.argsort(-scores, kind="stable")
    cum = np.cumsum(scores[order])
    thr = max(int((cum <= np.float32(GAMMA)).sum()), 1)
    selected = order[:thr]
    offs = np.array([[i, j] for i in (-1, 0, 1) for j in (-1, 0, 1)
                     if not (i == 0 and j == 0)])
    r = np.clip(selected[:, None] // GRID + offs[None, :, 0], 0, GRID - 1)
    c = np.clip(selected[:, None] % GRID + offs[None, :, 1], 0, GRID - 1)
    uniq = np.unique((r * GRID + c).reshape(-1))
    remained = np.setdiff1d(np.arange(LV), uniq)
    return thr, uniq, remained


def _shuffle(m):
    """[K, N] -> [128, K//128, N] so device tile [:, t, :] = rows t*128..t*128+128."""
    k, n = m.shape
    return np.ascontiguousarray(m.reshape(k // 128, 128, n).transpose(1, 0, 2))


def _pad_t(x, lp):
    """x [L, D] -> shuffled transpose [128, 32, lp] (zero-padded columns)."""
    out = np.zeros((D, lp), x.dtype)
    out[:, : x.shape[0]] = x.T
    return _shuffle(out)


def _colsum_tile(w):
    """w [F, D] bf16 -> [128, F//128] f32 column-sum tile ([p, m] = sum_d w[m*128+p])."""
    s = w.astype(np.float32).sum(1)
    return np.ascontiguousarray(s.reshape(-1, 128).T)


# ---------------------------------------------------------------- bass builder
def _build(lc, lr, ncu, ncr):
    from contextlib import ExitStack
    import concourse.bass as bass
    import concourse.tile as tile
    from concourse import bacc, mybir

    BF = mybir.dt.bfloat16
    F32 = mybir.dt.float32
    AF = mybir.ActivationFunctionType
    RG = [list(range(NCORES))]

    nc = bacc.Bacc("TRN2", target_bir_lowering=False, debug=False,
                   num_devices=NCORES)

    catT = nc.dram_tensor("catT", [128, NT, lc], BF, kind="ExternalInput").ap()
    remT = nc.dram_tensor("remT", [128, NT, lr], BF, kind="ExternalInput").ap()
    wqkv = [nc.dram_tensor(f"wqkv{i}", [16, 128, 4, 768], BF,
                           kind="ExternalInput").ap() for i in range(2)]
    wqkv3 = nc.dram_tensor("wqkv2", [24, 128, 4, 512], BF,
                           kind="ExternalInput").ap()
    wo = [nc.dram_tensor(f"wo{i}", [8, 128, 4, 512], BF,
                         kind="ExternalInput").ap() for i in range(3)]
    w1t = nc.dram_tensor("w1t", [8, 128, 16, 256], BF, kind="ExternalInput").ap()
    w2t = nc.dram_tensor("w2t", [8, 128, 8, 512], BF, kind="ExternalInput").ap()
    spt = nc.dram_tensor("spt", [128, NT, 1], BF, kind="ExternalInput").ap()
    eb_cat = nc.dram_tensor("eb_cat", [128, 1], F32, kind="ExternalInput").ap()
    eb_rem = nc.dram_tensor("eb_rem", [128, 1], F32, kind="ExternalInput").ap()
    sq3_d = nc.dram_tensor("sq3", [128, 4], F32, kind="ExternalInput").ap()
    sw1_d = nc.dram_tensor("sw1", [128, 8], F32, kind="ExternalInput").ap()
    spz_d = nc.dram_tensor("spz", [1, ncr], F32, kind="ExternalOutput").ap()
    r4_d = nc.dram_tensor("r4", [1, ncr], F32, kind="ExternalOutput").ap()
    nm4_d = nc.dram_tensor("nm4", [1, ncr], F32, kind="ExternalOutput").ap()

    with tile.TileContext(nc) as tc, ExitStack() as ctx:
        sb = ctx.enter_context(tc.tile_pool(name="sb", bufs=1))
        ws = ctx.enter_context(tc.tile_pool(name="ws", bufs=3))
        tp = ctx.enter_context(tc.tile_pool(name="tp", bufs=2))
        ps = ctx.enter_context(tc.tile_pool(name="ps", bufs=6, space="PSUM"))
        pst = ctx.enter_context(tc.tile_pool(name="pst", bufs=2, space="PSUM"))
        dr = ctx.enter_context(tc.tile_pool(name="dr", bufs=1, space="DRAM"))

        ones_bf = sb.tile([128, 1], BF, tag="ones", name="ones_bf")
        nc.vector.memset(ones_bf[:], 1.0)
        ones_row = sb.tile([1, 128], F32, tag="onesr", name="ones_row")
        nc.vector.memset(ones_row[:], 1.0)
        eps_t = sb.tile([1, 1], F32, tag="eps", name="eps_t")
        nc.vector.memset(eps_t[:], 1e-5)

        cat_sb = sb.tile([128, NT, lc], BF, tag="actC", name="cat_sb")
        nc.sync.dma_start(cat_sb[:], catT[:])
        rem_sb = sb.tile([128, NT, lr], BF, tag="actA", name="rem_sb")
        ebc_sb = sb.tile([128, 1], F32, tag="ebc", name="ebc_sb")
        nc.sync.dma_start(ebc_sb[:], eb_cat[:])
        ebr_sb = sb.tile([128, 1], F32, tag="ebr", name="ebr_sb")
        nc.sync.dma_start(ebr_sb[:], eb_rem[:])
        sq3_sb = sb.tile([128, 4], F32, tag="sq3", name="sq3_sb")
        nc.sync.dma_start(sq3_sb[:], sq3_d[:])
        sw1_sb = sb.tile([128, 8], F32, tag="sw1", name="sw1_sb")
        nc.sync.dma_start(sw1_sb[:], sw1_d[:])

        def bcast(row_f32, lq, nm):
            """[1, lq] f32 -> psum [128, lq] f32 via K=1 outer-product matmul."""
            pb = ps.tile([128, lq], F32, tag="pbig", name=f"bc{nm}")
            nc.tensor.matmul(pb[:], ones_row[:], row_f32, start=True, stop=True)
            return pb

        def ar_pair(lq, nch, nm):
            tpc = NT // nch
            ins_ = [dr.tile([128, tpc, lq], BF, tag=f"ai{nm}{g}", name=f"ai{nm}{g}")
                    for g in range(nch)]
            outs_ = [dr.tile([128, tpc, lq], BF, tag=f"ao{nm}{g}", name=f"ao{nm}{g}",
                             addr_space="Shared")
                     for g in range(nch)]
            return ins_, outs_

        def stage_and_reduce(t, lq, pps, arins, arouts, nm):
            """Copy psum tile t into the staging buffer; every 4 tiles DMA to the
            AR chunk buffer; when a chunk completes, launch its AllReduce."""
            tpc = NT // len(arins)
            g, t4 = t // 4, t % 4
            if t4 == 0:
                stage_and_reduce.cur = tp.tile([128, 4, lq], BF, tag="abig",
                                               bufs=2, name=f"ab{nm}{g}")
            nc.scalar.copy(stage_and_reduce.cur[:, t4, :], pps[:])
            if t4 == 3:
                c = t // tpc
                off = (g % (tpc // 4)) * 4
                nc.sync.dma_start(arins[c][:, off:off + 4, :],
                                  stage_and_reduce.cur[:])
                if t == (c + 1) * tpc - 1:
                    nc.gpsimd.collective_compute(
                        "AllReduce", mybir.AluOpType.add, replica_groups=RG,
                        ins=[arins[c].opt()], outs=[arouts[c].opt()])

        def attention_and_outproj(widx, qT, kT, vv, lq, lkp, eb_sb, wo_d):
            nlk = lkp // 128
            # ---- attention per head (softmax without max-subtraction)
            oT = tp.tile([128, 4, lq], BF, tag="oT", bufs=1, name=f"oT{widx}")
            for h in range(NH_CORE):
                expT = tp.tile([128, nlk, lq], BF, tag="expT", bufs=1,
                               name=f"expT{widx}_{h}")
                for lkt in range(nlk):
                    sps = ps.tile([128, lq], F32, tag="pbig", name=f"psc{widx}{h}{lkt}")
                    for td in range(2):
                        nc.tensor.matmul(sps[:],
                                         kT[:, h * 2 + td, lkt * 128:(lkt + 1) * 128],
                                         qT[:, h * 2 + td, :],
                                         start=(td == 0), stop=(td == 1))
                    bias = eb_sb[:] if lkt == nlk - 1 else 0.0
                    nc.scalar.activation(expT[:, lkt, :], sps[:], AF.Exp,
                                         scale=1.0 / 16.0, bias=bias)
                dps = pst.tile([1, lq], F32, tag="pstat", name=f"pd{widx}{h}")
                for lkt in range(nlk):
                    nc.tensor.matmul(dps[:], ones_bf[:], expT[:, lkt, :],
                                     start=(lkt == 0), stop=(lkt == nlk - 1))
                rc = tp.tile([1, lq], F32, tag="recip", bufs=1, name=f"rc{widx}{h}")
                nc.vector.reciprocal(rc[:], dps[:])
                rbp = bcast(rc[:], lq, f"r{widx}{h}")
                rbs = tp.tile([128, lq], F32, tag="rbs", bufs=1, name=f"rbs{widx}{h}")
                nc.scalar.copy(rbs[:], rbp[:])
                for td in range(2):
                    ops_ = ps.tile([128, lq], F32, tag="pbig", name=f"po{widx}{h}{td}")
                    for lkt in range(nlk):
                        nc.tensor.matmul(ops_[:],
                                         vv[:, lkt, h * 256 + td * 128:h * 256 + (td + 1) * 128],
                                         expT[:, lkt, :],
                                         start=(lkt == 0), stop=(lkt == nlk - 1))
                    nc.vector.tensor_mul(oT[:, h * 2 + td, :], ops_[:], rbs[:])
            # ---- out projection (row-parallel) + chunked AllReduce
            arins, arouts = ar_pair(lq, {0: 1, 1: 2, 2: 2}[widx], f"m{widx}")
            for ci in range(8):
                ch = ws.tile([128, 4, 512], BF, tag="wsmall", bufs=3, name=f"woc{widx}{ci}")
                nc.sync.dma_start(ch[:], wo_d[ci])
                for tl in range(4):
                    t = ci * 4 + tl
                    pps = ps.tile([128, lq], F32, tag="pbig", name=f"pop{widx}{t}")
                    for td in range(4):
                        nc.tensor.matmul(pps[:], ch[:, td, tl * 128:(tl + 1) * 128],
                                         oT[:, td, :], start=(td == 0), stop=(td == 3))
                    stage_and_reduce(t, lq, pps, arins, arouts, f"m{widx}")
            return arouts

        def mha(widx, xq, lq, xkv, lkp, eb_sb):
            """One TP-sharded MHA block; returns chunked AllReduce output tiles.

            xq: [128, NT, >=lq] tile (q-side rhs sliced to exact lq).
            xkv: [128, NT, lkp] tile (k/v side, lkp padded to x128, eb masks pad).
            """
            nlk = lkp // 128
            qT = tp.tile([128, 4, lq], BF, tag="qT", bufs=1, name=f"qT{widx}")
            kT = tp.tile([128, 4, lkp], BF, tag="kT", bufs=1, name=f"kT{widx}")
            vv = tp.tile([128, nlk, DHC], BF, tag="vv", bufs=1, name=f"vv{widx}")
            # ---- fused QKV projection, weight-streamed in two column groups
            for grp in (0, 1):
                if grp == 0:  # cols 0:768 -> q0..q3, k0, k1
                    pls = [ps.tile([128, lq], F32, tag="pbig",
                                   name=f"pq{widx}_{m}") for m in range(4)]
                    pls += [ps.tile([128, lkp], F32, tag="pbig",
                                    name=f"pk{widx}_{m}") for m in range(2)]
                else:  # cols 768:1536 -> k2, k3, v rows
                    pls = [ps.tile([128, lkp], F32, tag="pbig",
                                   name=f"pk{widx}_{2 + m}") for m in range(2)]
                    pls += [ps.tile([128, DHC], F32, tag="pbig",
                                    name=f"pv{widx}_{m}") for m in range(nlk)]
                for kc in range(8):
                    ch = ws.tile([128, 4, 768], BF, tag="wqkvch", bufs=4, name=f"wc{widx}{grp}{kc}")
                    nc.sync.dma_start(ch[:], wqkv[widx][grp * 8 + kc])
                    for t8 in range(4):
                        t = kc * 4 + t8
                        st, sp_ = (t == 0), (t == 31)
                        if grp == 0:
                            for m in range(4):
                                nc.tensor.matmul(pls[m][:], ch[:, t8, m * 128:(m + 1) * 128],
                                                 xq[:, t, 0:lq], start=st, stop=sp_)
                            for m in range(2):
                                nc.tensor.matmul(pls[4 + m][:],
                                                 ch[:, t8, 512 + m * 128:512 + (m + 1) * 128],
                                                 xkv[:, t, :], start=st, stop=sp_)
                        else:
                            for m in range(2):
                                nc.tensor.matmul(pls[m][:], ch[:, t8, m * 128:(m + 1) * 128],
                                                 xkv[:, t, :], start=st, stop=sp_)
                            for mk in range(nlk):
                                nc.tensor.matmul(pls[2 + mk][:],
                                                 xkv[:, t, mk * 128:(mk + 1) * 128],
                                                 ch[:, t8, 256:768], start=st, stop=sp_)
                if grp == 0:
                    for m in range(4):
                        nc.scalar.copy(qT[:, m, :], pls[m][:])
                    for m in range(2):
                        nc.scalar.copy(kT[:, m, :], pls[4 + m][:])
                else:
                    for m in range(2):
                        nc.scalar.copy(kT[:, 2 + m, :], pls[m][:])
                    for mk in range(nlk):
                        nc.scalar.copy(vv[:, mk, :], pls[2 + mk][:])
            arouts = attention_and_outproj(widx, qT, kT, vv, lq, lkp, eb_sb,
                                            wo[widx])
            return arouts

        def ln(base, arouts, lq, out_tag, out_name, lpad=None, mode="inplace",
               fused_base=None, extra_mm=None, want_fix=False, sep_tag=None,
               hook=None):
            """z = base + ar (optionally base = z_pre*rb + nb fused from a
            deferred LN); stats accumulate per arriving AllReduce chunk.
            mode: "inplace" (normalize z in place), "separate" (keep z pre-norm,
            write normalized copy to sep_tag tile), "defer" (keep z pre-norm,
            return bf16+f32 row broadcasts for downstream fixup/fusion),
            "none" (z transient, stats only).
            Returns (z, applied, r, nmr, rb16, nb16, rb_s, nmrb_s)."""
            z = None
            if mode != "none":
                zw = lpad if lpad is not None else lq
                z = sb.tile([128, NT, zw], BF, tag=out_tag, name=out_name)
                if zw > lq:
                    nc.vector.memset(z[:, :, lq:zw], 0.0)
            sums = pst.tile([1, lq], F32, tag="pstat", name=f"su{out_name}")
            sqs = pst.tile([1, lq], F32, tag="pstat", name=f"sq{out_name}")
            tpc = NT // len(arouts)
            for g in range(NT // 4):
                arB = tp.tile([128, 4, lq], BF, tag="arB", bufs=2,
                              name=f"arB{out_name}{g}")
                c = (g * 4) // tpc
                off = (g * 4) % tpc
                nc.sync.dma_start(arB[:], arouts[c][:, off:off + 4, :])
                for t4 in range(4):
                    t = g * 4 + t4
                    if mode != "none":
                        zt = z[:, t, 0:lq]
                    else:
                        ztile = tp.tile([128, lq], BF, tag="z4t", bufs=2,
                                        name=f"zt{out_name}{t}")
                        zt = ztile[:]
                    if fused_base is not None:
                        zp, frb, fnb = fused_base
                        fz = tp.tile([128, lq], BF, tag="lnt", bufs=2,
                                     name=f"fz{out_name}{t}")
                        nc.vector.tensor_mul(fz[:], zp[:, t, 0:lq], frb[:])
                        nc.vector.tensor_add(fz[:], fz[:], fnb[:])
                        nc.vector.tensor_add(zt, fz[:], arB[:, t4, :])
                    else:
                        nc.vector.tensor_add(zt, base[:, t, 0:lq], arB[:, t4, :])
                    nc.tensor.matmul(sums[:], ones_bf[:], zt,
                                     start=(t == 0), stop=(t == NT - 1))
                    sq = tp.tile([128, lq], BF, tag="sq", bufs=2,
                                 name=f"q{out_name}{t}")
                    nc.vector.tensor_mul(sq[:], zt, zt)
                    nc.tensor.matmul(sqs[:], ones_bf[:], sq[:],
                                     start=(t == 0), stop=(t == NT - 1))
                    if extra_mm is not None:
                        extra_mm(t, zt)
                    if hook is not None:
                        hook(t, zt)
            mu = tp.tile([1, lq], F32, tag="lns", bufs=4, name=f"mu{out_name}")
            nc.scalar.mul(mu[:], sums[:], 1.0 / D)
            ex2 = tp.tile([1, lq], F32, tag="lns", bufs=4, name=f"e2{out_name}")
            nc.scalar.mul(ex2[:], sqs[:], 1.0 / D)
            tmp = tp.tile([1, lq], F32, tag="lns", bufs=4, name=f"va{out_name}")
            nc.vector.tensor_mul(tmp[:], mu[:], mu[:])
            nc.vector.tensor_sub(tmp[:], ex2[:], tmp[:])
            nc.scalar.activation(tmp[:], tmp[:], AF.Sqrt, bias=eps_t[:])
            r_ = tp.tile([1, lq], F32, tag="lns", bufs=4, name=f"r{out_name}")
            nc.vector.reciprocal(r_[:], tmp[:])
            nmr = mu
            nc.vector.tensor_mul(nmr[:], nmr[:], r_[:])
            nc.scalar.mul(nmr[:], nmr[:], -1.0)
            if mode == "none":
                return None, None, r_, nmr, None, None, None, None
            rbp = bcast(r_[:], lq, f"lr{out_name}")
            rb16 = tp.tile([128, lq], BF, tag="lnb", bufs=2, name=f"rb{out_name}")
            nc.scalar.copy(rb16[:], rbp[:])
            rb_s = None
            if want_fix:
                rb_s = tp.tile([128, lq], F32, tag="lnbf", bufs=2,
                               name=f"rf{out_name}")
                nc.scalar.copy(rb_s[:], rbp[:])
            nbp = bcast(nmr[:], lq, f"ln{out_name}")
            nb16 = tp.tile([128, lq], BF, tag="lnb", bufs=2, name=f"nb{out_name}")
            nc.scalar.copy(nb16[:], nbp[:])
            nmrb_s = None
            if want_fix:
                nmrb_s = tp.tile([128, lq], F32, tag="lnbf", bufs=2,
                                 name=f"nf{out_name}")
                nc.scalar.copy(nmrb_s[:], nbp[:])
            applied = None
            if mode == "inplace":
                for t in range(NT):
                    tm = tp.tile([128, lq], BF, tag="lnt", bufs=2,
                                 name=f"tm{out_name}{t}")
                    nc.vector.tensor_mul(tm[:], z[:, t, 0:lq], rb16[:])
                    nc.vector.tensor_add(z[:, t, 0:lq], tm[:], nb16[:])
                applied = z
            elif mode == "separate":
                applied = sb.tile([128, NT, lq], BF, tag=sep_tag,
                                  name=f"{out_name}_ap")
                for t in range(NT):
                    tm = tp.tile([128, lq], BF, tag="lnt", bufs=2,
                                 name=f"tm{out_name}{t}")
                    nc.vector.tensor_mul(tm[:], z[:, t, 0:lq], rb16[:])
                    nc.vector.tensor_add(applied[:, t, :], tm[:], nb16[:])
            return z, applied, r_, nmr, rb16, nb16, rb_s, nmrb_s

        # branch A (cat) and branch B (rem) are independent up to MHA3
        ar1 = mha(0, cat_sb, ncu, cat_sb, lc, ebc_sb)
        nc.sync.dma_start(rem_sb[:], remT[:])
        ar2 = mha(1, rem_sb, ncr, rem_sb, lr, ebr_sb)
        # x: applied LN1 output, padded to lc for use as MHA3's k/v side
        x_bf, _, _, _, _, _, _, _ = ln(cat_sb, ar1, ncu, "actD", "x_bf",
                                       lpad=lc, mode="inplace")
        x_bf = x_bf  # applied in place

        # ---- MHA3 K/V projection on x (early, independent of AR2)
        nlk3 = lc // 128
        kT3 = tp.tile([128, 4, lc], BF, tag="kT", bufs=1, name="kT3")
        vv3 = tp.tile([128, nlk3, DHC], BF, tag="vv", bufs=1, name="vv3")
        for grp in (1, 2):
            if grp == 1:
                pls3 = [ps.tile([128, lc], F32, tag="pbig", name=f"pk2_{m}")
                        for m in range(4)]
            else:
                pls3 = [ps.tile([128, DHC], F32, tag="pbig", name=f"pv2_{m}")
                        for m in range(nlk3)]
            for kc in range(8):
                ch = ws.tile([128, 4, 512], BF, tag="wqkvch", bufs=4,
                             name=f"wc3{grp}{kc}")
                nc.sync.dma_start(ch[:], wqkv3[grp * 8 + kc])
                for t8 in range(4):
                    t = kc * 4 + t8
                    st, sp_ = (t == 0), (t == 31)
                    if grp == 1:
                        for m in range(4):
                            nc.tensor.matmul(pls3[m][:], ch[:, t8, m * 128:(m + 1) * 128],
                                             x_bf[:, t, :], start=st, stop=sp_)
                    else:
                        for mk in range(nlk3):
                            nc.tensor.matmul(pls3[mk][:],
                                             x_bf[:, t, mk * 128:(mk + 1) * 128],
                                             ch[:, t8, :], start=st, stop=sp_)
            if grp == 1:
                for m in range(4):
                    nc.scalar.copy(kT3[:, m, :], pls3[m][:])
            else:
                for mk in range(nlk3):
                    nc.scalar.copy(vv3[:, mk, :], pls3[mk][:])

        # ---- LN3 (deferred) with MHA3's Q-projection fused into the chunk loop
        q3 = {}

        def q3_hook(t, zt):
            if t == 0:
                q3["p"] = [ps.tile([128, ncr], F32, tag="pbig", name=f"pq2_{m}")
                           for m in range(4)]
            if t % 4 == 0:
                q3["ch"] = ws.tile([128, 4, 512], BF, tag="wqkvch", bufs=4,
                                   name=f"wcq3{t // 4}")
                nc.sync.dma_start(q3["ch"][:], wqkv3[t // 4])
            for m in range(4):
                nc.tensor.matmul(q3["p"][m][:],
                                 q3["ch"][:, t % 4, m * 128:(m + 1) * 128],
                                 zt, start=(t == 0), stop=(t == NT - 1))

        z2, _, _, _, rb16_3, nb16_3, rb3, nf3 = ln(
            rem_sb, ar2, ncr, "actB", "z2_bf", mode="defer", want_fix=True,
            hook=q3_hook)
        qT3 = tp.tile([128, 4, ncr], BF, tag="qT", bufs=1, name="qT3")
        for m in range(4):
            f1 = tp.tile([128, ncr], F32, tag="fixt", bufs=2, name=f"f1q3{m}")
            nc.vector.tensor_mul(f1[:], q3["p"][m][:], rb3[:])
            f2 = tp.tile([128, ncr], F32, tag="fixt", bufs=2, name=f"f2q3{m}")
            nc.vector.tensor_scalar(
                out=f2[:], in0=nf3[:], scalar1=sq3_sb[:, m:m + 1],
                scalar2=None, op0=mybir.AluOpType.mult)
            nc.vector.tensor_add(qT3[:, m, :], f1[:], f2[:])
        ar3 = attention_and_outproj(2, qT3, kT3, vv3, ncr, lc, ebc_sb, wo[2])

        # ---- LN2 with FFN w1 wave-A (hid tiles 0..3) fused into the chunk loop
        hT = sb.tile([128, HIDC // 128, ncr], BF, tag="hT", name="hT")
        w1a = {}

        def w1a_hook(t, zt):
            if t == 0:
                w1a["p"] = [ps.tile([128, ncr], F32, tag="pbig", name=f"ph_{m}")
                            for m in range(4)]
            if t % 16 == 0:
                kc = t // 16
                w1a["ch"] = [ws.tile([128, 16, 256], BF, tag="wsmall", bufs=3,
                                     name=f"w1a{mp}{kc}") for mp in range(2)]
                for mp in range(2):
                    nc.sync.dma_start(w1a["ch"][mp][:], w1t[mp * 2 + kc])
            for mp in range(2):
                for ml in range(2):
                    nc.tensor.matmul(w1a["p"][mp * 2 + ml][:],
                                     w1a["ch"][mp][:, t % 16, ml * 128:(ml + 1) * 128],
                                     zt, start=(t == 0), stop=(t == NT - 1))

        z3, x2_bf, _, _, _, _, rb2, nf2 = ln(
            None, ar3, ncr, "actD", "z3_bf", mode="separate", sep_tag="actA",
            fused_base=(z2, rb16_3, nb16_3), want_fix=True, hook=w1a_hook)

        def w1_fix(m, psrc):
            f1 = tp.tile([128, ncr], F32, tag="fixt", bufs=2, name=f"f1h{m}")
            nc.vector.tensor_mul(f1[:], psrc[:], rb2[:])
            f2 = tp.tile([128, ncr], F32, tag="fixt", bufs=2, name=f"f2h{m}")
            nc.vector.tensor_scalar(
                out=f2[:], in0=nf2[:], scalar1=sw1_sb[:, m:m + 1],
                scalar2=None, op0=mybir.AluOpType.mult)
            nc.vector.tensor_add(f1[:], f1[:], f2[:])
            nc.scalar.activation(hT[:, m, :], f1[:], AF.Gelu)

        for m in range(4):
            w1_fix(m, w1a["p"][m])
        # wave B (hid tiles 4..7) on the completed z3
        for mp in (2, 3):
            plsb = [ps.tile([128, ncr], F32, tag="pbig", name=f"phb{mp}_{m}")
                    for m in range(2)]
            for kc in range(2):
                ch = ws.tile([128, 16, 256], BF, tag="wsmall", bufs=3,
                             name=f"w1b{mp}{kc}")
                nc.sync.dma_start(ch[:], w1t[mp * 2 + kc])
                for t16 in range(16):
                    t = kc * 16 + t16
                    for ml in range(2):
                        nc.tensor.matmul(plsb[ml][:], ch[:, t16, ml * 128:(ml + 1) * 128],
                                         z3[:, t, :], start=(t == 0), stop=(t == 31))
            for ml in range(2):
                w1_fix(mp * 2 + ml, plsb[ml])
        ar4ins, ar4outs = ar_pair(ncr, 2, "f")
        for ci in range(8):
            ch = ws.tile([128, 8, 512], BF, tag="wsmall", bufs=3, name=f"w2c{ci}")
            nc.sync.dma_start(ch[:], w2t[ci])
            for tl in range(4):
                t = ci * 4 + tl
                pps = ps.tile([128, ncr], F32, tag="pbig", name=f"pw2{t}")
                for th in range(8):
                    nc.tensor.matmul(pps[:], ch[:, th, tl * 128:(tl + 1) * 128],
                                     hT[:, th, :], start=(th == 0), stop=(th == 7))
                stage_and_reduce(t, ncr, pps, ar4ins, ar4outs, "f")

        # ---- LN4 stats + scorer matmul on pre-norm z (normalized on host)
        sp_sb = sb.tile([128, NT, 1], BF, tag="spt", name="sp_sb")
        nc.sync.dma_start(sp_sb[:], spt[:])
        lps = ps.tile([1, ncr], F32, tag="pbig", name="lps")

        def spz_mm(t, zt):
            nc.tensor.matmul(lps[:], sp_sb[:, t, :], zt,
                             start=(t == 0), stop=(t == NT - 1))

        _, _, r4, nm4, _, _, _, _ = ln(x2_bf, ar4outs, ncr, "", "z4",
                                       mode="none", extra_mm=spz_mm)
        lg = tp.tile([1, ncr], F32, tag="lns", bufs=4, name="lg")
        nc.vector.tensor_copy(lg[:], lps[:])
        nc.sync.dma_start(spz_d[:], lg[:])
        nc.sync.dma_start(r4_d[:], r4[:])
        nc.sync.dma_start(nm4_d[:], nm4[:])

    nc.compile()
    return nc


# ---------------------------------------------------------------- entry point
def kernel(**inputs):
    global LAST_EXEC_NS
    vf = np.asarray(inputs["vision_feature"], np.float32)
    te = np.asarray(inputs["text_embed"], np.float32)
    mask = np.asarray(inputs["attention_mask"])

    thr, uniq, remained = _route_np(vf, te, mask)
    cat = np.concatenate([vf[uniq], te], 0)
    rem = vf[remained]
    ncu, ncr = cat.shape[0], rem.shape[0]
    lc = -(-ncu // 128) * 128
    lr = -(-ncr // 128) * 128

    key = (lc, lr, ncu, ncr)
    if key not in _CACHE:
        _CACHE[key] = _build(*key)
    nc = _CACHE[key]

    catT = _pad_t(cat.astype(BF16), lc)
    remT = _pad_t(rem.astype(BF16), lr)

    def _eb(nvalid, lpad):
        v = nvalid - (lpad // 128 - 1) * 128
        b = np.zeros((128, 1), np.float32)
        b[v:] = -1e5
        return b

    eb_cat = _eb(ncu, lc)
    eb_rem = _eb(ncr, lr)

    in_maps = []
    for c in range(NCORES):
        hs = slice(c * DHC, (c + 1) * DHC)
        m = {"catT": catT, "remT": remT, "eb_cat": eb_cat, "eb_rem": eb_rem,
             "spt": _shuffle(np.ascontiguousarray(
                 np.asarray(inputs["sp_w"], np.float32).T.reshape(D, 1).astype(BF16)))}
        for i, w in enumerate(("sa1_w", "sa2_w", "ca_w")):
            win = np.asarray(inputs[w], np.float32)
            wq, wk, wv = win[:D][hs], win[D:2 * D][hs], win[2 * D:][hs]
            sh = _shuffle(np.ascontiguousarray(
                np.concatenate([wq.T, wk.T, wv.T], 1)).astype(BF16))
            if w == "ca_w":
                m["wqkv2"] = np.stack([
                    sh[:, kc * 4:(kc + 1) * 4, grp * 512:(grp + 1) * 512]
                    for grp in range(3) for kc in range(8)])
                m["sq3"] = _colsum_tile(wq.astype(BF16))
            else:
                m[f"wqkv{i}"] = np.stack([
                    sh[:, kc * 4:(kc + 1) * 4, grp * 768:(grp + 1) * 768]
                    for grp in range(2) for kc in range(8)])
        for i, w in enumerate(("sa1_ow", "sa2_ow", "ca_ow")):
            wout = np.asarray(inputs[w], np.float32)
            sh = _shuffle(np.ascontiguousarray(wout[:, hs].T).astype(BF16))
            m[f"wo{i}"] = np.stack([sh[:, :, ci * 512:(ci + 1) * 512]
                                    for ci in range(8)])
        w1c = np.asarray(inputs["ffn_w1"], np.float32)[c * HIDC:(c + 1) * HIDC]
        m["sw1"] = _colsum_tile(w1c.astype(BF16))
        sh = _shuffle(np.ascontiguousarray(w1c.T).astype(BF16))
        m["w1t"] = np.stack([sh[:, kc * 16:(kc + 1) * 16, mp * 256:(mp + 1) * 256]
                             for mp in range(4) for kc in range(2)])
        sh = _shuffle(np.ascontiguousarray(
            np.asarray(inputs["ffn_w2"], np.float32)[:, c * HIDC:(c + 1) * HIDC].T
        ).astype(BF16))
        m["w2t"] = np.stack([sh[:, :, ci * 512:(ci + 1) * 512] for ci in range(8)])
        in_maps.append(m)

    from concourse import bass_utils
    res = bass_utils.run_bass_kernel_spmd(nc, in_maps, core_ids=list(range(NCORES)))
    LAST_EXEC_NS = res.exec_time_ns

    rr = res.results[0]
    spz = np.asarray(rr["spz"], np.float32)[0]
    r4 = np.asarray(rr["r4"], np.float32)[0]
    nm4 = np.asarray(rr["nm4"], np.float32)[0]
    s_sp = np.float32(np.asarray(inputs["sp_w"], np.float32)
                      .astype(BF16).astype(np.float32).sum())
    logit = r4 * spz + s_sp * nm4 + np.float32(inputs["sp_b"][0])
    k = max(int(thr * EXPAND_RATIO), 1)
    gi = np.argsort(-logit, kind="stable")[:k]
    final = np.unique(np.concatenate([uniq, remained[gi]]))
    return vf[final]
